# revision 22
# baseline (speedup 1.0000x reference)
"""GCN (3-layer, PyG-style) forward on 8 Trainium2 NeuronCores.

Math restructuring
------------------
reference:
  h1 = relu(Anorm @ x @ W1 + b1)          (Anorm includes self loops + sym norm)
  h2 = relu(Anorm @ h1 @ W2 + b2)
  h3 = Anorm @ h2 @ W3 + b3
  out = segment_mean(h3, batch) @ Wlin + blin

Because GCNConv aggregation and the weight matmul commute, and pooling is
linear, this is equivalent to:
  agg1 = Anorm @ x                        # [N,2]  (tiny -> host)
  msg_e = relu(norm_e * (agg1[src_e] @ W1 + b1))    # per-edge (norm>0 commutes
                                                    #  through relu)
  g2   = scatter-sum msg to dst           # exact one-hot matmul on device
  h2   = relu(g2 @ W2 + b2)               # dense matmul on device
  pg3[g] = sum_n T[n,g] * h2[n]           # T[n,g] = sum of norm over n's
                                          #  out-edges into graph g
  out  = ((pg3 @ W3 + cnt*b3)/max(cnt,1)) @ Wlin + blin   # [128,1024] -> host

Sharding: nodes are LPT bin-packed into 8 cores x 98 windows of 128 slots so
that each (core, window) bin holds ~638 incident edges (load-balanced).  Every
core runs the same program (SPMD) on its own edge arrays, padded to identical
tile counts.  Per-core output is a partial pg3 [128,1024]; the host sums them
(the "all-reduce").

Device-side structure:
 - L1 (K=4, bf16) matmuls are 4-way row-group packed via tile_position: aT and
   W1b are duplicated at SBUF base partitions {0,32,64,96} so the four matmuls
   of an edge-tile pair run concurrently in separate 32-row strips
 - messages are stored fp8e4m3 (x16); the aggregation runs in "dual form"
   (g2T[f,dst] += msg[e,f].T @ S[e,dst]) as fp8 DoubleRow matmuls that pack
   two edge tiles per instruction; S is an exact one-hot so a window's odd
   tail tile is paired with an all-zero S column (masking a dummy slot)
 - h2 = relu(g2 @ W2 + b2) also runs as fp8 DoubleRow (g2T x8, W2 x16,
   rescaled 1/128 inside the relu); pg3 accumulation is bf16->fp32
 - L1 work for window w+1 is interleaved between the aggregation passes of
   window w, keeping every matmul's dependencies one window ahead (dense PE
   stream, HAM stays warm) and covering the PSUM drain latencies
"""

import numpy as np

LAST_RESULTS = None  # set by kernel() for test harness introspection

N_NODES = 100000
N_EDGES = 400000
G = 128
FIN = 2
H = 1024
N_CORES = 8
P = 128
NW = 98                      # windows per core (98*128 = 12544 >= 12500 slots)
NBINS = N_CORES * NW


def _lpt_pack(wgt):
    """Assign each node to one of 784 (core,window) bins, balancing total
    edge weight per bin with a <=128 nodes/bin cap.  Returns (bin_of, slot_of).
    """
    import heapq

    n = len(wgt)
    order = np.argsort(-wgt, kind="stable")
    heap = [(0, 0, b) for b in range(NBINS)]
    heapq.heapify(heap)
    bin_of = np.empty(n, dtype=np.int64)
    slot_of = np.empty(n, dtype=np.int64)
    w_arr = wgt.tolist()
    for idx in order.tolist():
        while True:
            load, count, b = heapq.heappop(heap)
            if count < P:
                break
        bin_of[idx] = b
        slot_of[idx] = count
        heapq.heappush(heap, (load + w_arr[idx], count + 1, b))
    return bin_of, slot_of


def _host_prep(x, edge_index, batch):
    """All O(E) index work in numpy; returns per-core device arrays."""
    import ml_dtypes

    bf16 = ml_dtypes.bfloat16
    fp8 = ml_dtypes.float8_e4m3
    x = np.asarray(x, dtype=np.float32)
    ei = np.asarray(edge_index).astype(np.int64)
    batch = np.asarray(batch).astype(np.int64)
    n = N_NODES

    loops = np.arange(n, dtype=np.int64)
    row = np.concatenate([ei[0], loops])
    col = np.concatenate([ei[1], loops])

    deg = np.bincount(col, minlength=n).astype(np.float64)
    dis = np.where(deg > 0, 1.0 / np.sqrt(np.maximum(deg, 1.0)), 0.0)
    norm = dis[row] * dis[col]                     # fp64

    # layer-1 aggregation (FIN=2) on host
    agg1 = np.empty((n, FIN), dtype=np.float64)
    for f in range(FIN):
        agg1[:, f] = np.bincount(
            col, weights=norm * x[row, f].astype(np.float64), minlength=n
        )

    # ---- node -> (core, window, slot) via LPT packing on indegree+1 ----
    wgt = np.bincount(col, minlength=n)            # includes the self loop
    bin_raw, slot_of = _lpt_pack(wgt)
    # deal bins to (core, window) so similar loads share a window
    loads = np.zeros(NBINS, dtype=np.int64)
    np.add.at(loads, bin_raw, wgt)
    deal = np.argsort(-loads, kind="stable")       # deal[k] = raw bin id
    bin_rank = np.empty(NBINS, dtype=np.int64)
    bin_rank[deal] = np.arange(NBINS)
    rank = bin_rank[bin_raw]                       # 0..783, sorted by load
    node_w = rank // N_CORES                       # window 0..97
    node_c = rank % N_CORES                        # core 0..7

    # ---- edges ordered by (dst core, dst window) ----
    e_rank = rank[col]
    order = np.argsort(e_rank, kind="stable")
    row_s, col_s = row[order], col[order]
    norm_s = norm[order]
    rank_s = e_rank[order]
    c_s = rank_s % N_CORES
    w_s = rank_s // N_CORES

    cnts = np.bincount(e_rank, minlength=NBINS)    # indexed by rank = w*8 + c
    cw_load = cnts.reshape(NW, N_CORES).T          # [core, window]
    T_w = ((cw_load.max(axis=0) + P - 1) // P).astype(np.int64)   # per window
    base_tile = np.concatenate([[0], np.cumsum(T_w)])
    TT = int(base_tile[-1])

    starts = np.concatenate([[0], np.cumsum(cnts)])
    idx_in_bin = np.arange(len(col_s)) - starts[rank_s]
    tile_g = base_tile[w_s] + idx_in_bin // P
    slot = tile_g * P + idx_in_bin % P

    # flat per-slot arrays (norm folded into aT; S is exact one-hot)
    aT = np.zeros((N_CORES, 4, TT * P), dtype=np.float32)
    Sfull = np.zeros((N_CORES, TT * P, P), dtype=fp8)
    aT[c_s, 0, slot] = (agg1[row_s, 0] * norm_s).astype(np.float32)
    aT[c_s, 1, slot] = (agg1[row_s, 1] * norm_s).astype(np.float32)
    aT[c_s, 2, slot] = norm_s.astype(np.float32)
    Sfull[c_s, slot, slot_of[col_s]] = fp8(1.0)
    aTt = aT.reshape(N_CORES, 4, TT, P)
    Sfull = Sfull.reshape(N_CORES, TT, P, P)

    # ---- window-local tile pairing into "blocks" (2 tiles per block) ----
    # block kinds: 1 = full pair, 0 = tail (second slot S-masked to zero)
    nblk_w = ((T_w + 1) // 2).astype(np.int64)     # blocks per window
    base_blk = np.concatenate([[0], np.cumsum(nblk_w)])
    NBLK = int(base_blk[-1])

    aT4 = np.zeros((N_CORES, 16, NBLK * P), dtype=np.float32)
    S2 = np.zeros((N_CORES, NBLK, P, 2, P), dtype=fp8)
    blk_kind = np.zeros(NBLK, dtype=np.int64)
    for w in range(NW):
        nt = int(T_w[w])
        for b in range(int(nblk_w[w])):
            blk = int(base_blk[w]) + b
            t0 = int(base_tile[w]) + 2 * b
            csl = slice(blk * P, (blk + 1) * P)
            aT4[:, 0:4, csl] = aTt[:, :, t0]
            aT4[:, 8:12, csl] = aTt[:, :, t0]
            S2[:, blk, :, 0, :] = Sfull[:, t0]
            if 2 * b + 1 < nt:
                blk_kind[blk] = 1
                aT4[:, 4:8, csl] = aTt[:, :, t0 + 1]
                aT4[:, 12:16, csl] = aTt[:, :, t0 + 1]
                S2[:, blk, :, 1, :] = Sfull[:, t0 + 1]

    # ---- L3: T matrix rows permuted to node home slots ----
    gcol = batch[col]                              # graph of each edge's dst
    Tmat = np.bincount(
        row * G + gcol, weights=norm, minlength=n * G
    ).astype(np.float32).reshape(n, G)
    Tpad = np.zeros((N_CORES, NW * P, G), dtype=bf16)
    Tpad[node_c, node_w * P + slot_of] = Tmat.astype(bf16)

    cnt = np.bincount(batch, minlength=G).astype(np.float32)
    return (aT4.astype(bf16), S2, Tpad, cnt, nblk_w, blk_kind, NBLK, base_blk)


def _build_device_program(NBLK, nblk_w, blk_kind, base_blk, nw=NW):
    import concourse.mybir as mybir
    import concourse.tile as tile
    from concourse import bacc

    f32 = mybir.dt.float32
    bf16 = mybir.dt.bfloat16
    fp8 = mybir.dt.float8e4
    DR = mybir.MatmulPerfMode.DoubleRow
    nc = bacc.Bacc(None, target_bir_lowering=False, debug=False)

    aT_d = nc.dram_tensor("aT", [16, NBLK * P], bf16, kind="ExternalInput")
    S_d = nc.dram_tensor("S", [NBLK, P, 2, P], fp8, kind="ExternalInput")
    T_d = nc.dram_tensor("T", [NW, P, G], bf16, kind="ExternalInput")
    W1b_d = nc.dram_tensor("W1b", [4, H], bf16, kind="ExternalInput")
    W2_d = nc.dram_tensor("W2", [8, P, H], fp8, kind="ExternalInput")
    b2_d = nc.dram_tensor("b2", [1, H], bf16, kind="ExternalInput")
    out_d = nc.dram_tensor("pg3", [G, H], f32, kind="ExternalOutput")

    CH = 16                      # aT blocks per staged chunk
    n_chunks = (NBLK + CH - 1) // CH
    MS = 16.0                    # msg fp8 scale
    GS = 8.0                     # g2T fp8 scale

    with tile.TileContext(nc) as tc:
        with (
            tc.tile_pool(name="const", bufs=1) as cst,
            tc.tile_pool(name="sa", bufs=2) as sa,
            tc.tile_pool(name="sS", bufs=10) as sS,
            tc.tile_pool(name="smsg", bufs=8) as smsg,
            tc.tile_pool(name="stail", bufs=2) as stail,
            tc.tile_pool(name="sg2T", bufs=2) as sg2T,
            tc.tile_pool(name="sh2", bufs=2) as sh2,
            tc.tile_pool(name="sT", bufs=2) as sT,
            tc.tile_pool(name="zp", bufs=4, space="PSUM") as zp,
            tc.tile_pool(name="gp", bufs=2, space="PSUM") as gp,
            tc.tile_pool(name="hp", bufs=2, space="PSUM") as hp,
        ):
            Relu = mybir.ActivationFunctionType.Relu
            Copy = mybir.ActivationFunctionType.Copy
            Mult = mybir.AluOpType.mult
            Max = mybir.AluOpType.max

            # W1b duplicated at base partitions 0/32/64/96
            W1bd = cst.tile([100, H], bf16, tag="W1bd")
            for g4 in range(4):
                nc.sync.dma_start(W1bd[g4 * 32 : g4 * 32 + 4, :], W1b_d[:])
            W2s = cst.tile([P, 8, H], fp8, tag="W2s")
            nc.sync.dma_start(W2s[:], W2_d[:].rearrange("c p f -> p c f"))
            b2s = cst.tile([1, H], bf16, tag="b2s")
            nc.sync.dma_start(b2s[:], b2_d[:])
            ones1 = cst.tile([1, P], bf16, tag="ones1")
            nc.vector.memset(ones1[:], 1.0)
            pg3s = cst.tile([G, H], f32, tag="pg3s")
            nc.vector.memset(pg3s[:], 0.0)

            # dedicated tail msg tiles: slot 1 zeroed once (S masks it, but
            # the DoubleRow lhsT read must see finite values)
            tail_a = stail.tile([P, 2, H], fp8, tag="mt")
            tail_b = stail.tile([P, 2, H], fp8, tag="mt")
            tails = [tail_a, tail_b]
            for t_ in tails:
                nc.vector.memset(t_[:, 1, :], 0.0)

            chunks = {}          # chunk idx -> staged aT tile
            msg_of = {}          # block -> msg pair tile [P, 2, H]
            Ss_of = {}           # block -> one-hot S pair tile [P, 2, P]

            def stage_chunk(ci):
                if ci >= n_chunks or ci in chunks:
                    return
                t_ = sa.tile([100, CH * P], bf16, tag="aTc")
                lo = ci * CH * P
                hi = min((ci + 1) * CH * P, NBLK * P)
                for g4 in range(4):
                    nc.sync.dma_start(
                        t_[g4 * 32 : g4 * 32 + 4, : hi - lo],
                        aT_d[g4 * 4 : g4 * 4 + 4, lo:hi],
                    )
                chunks[ci] = t_

            state = {"b": 0, "tail": 0}

            def emit_block():
                blk = state["b"]
                if blk >= NBLK:
                    return
                state["b"] = blk + 1
                ci, off = blk // CH, (blk % CH) * P
                if blk % CH == 0:
                    stage_chunk(ci + 1)
                aTc = chunks[ci]
                full = blk_kind[blk] == 1
                Ss = sS.tile([P, 2, P], fp8, tag="Ss")
                nc.sync.dma_start(Ss[:], S_d[blk])
                Ss_of[blk] = Ss
                sl = slice(off, off + P)
                if full:
                    mp = smsg.tile([P, 2, H], fp8, tag="msg")
                    zAe = zp.tile([P, 512], f32, tag="z")
                    zAo = zp.tile([P, 512], f32, tag="z")
                    zBe = zp.tile([P, 512], f32, tag="z")
                    zBo = zp.tile([P, 512], f32, tag="z")
                    nc.tensor.matmul(zAe[:], aTc[0:4, sl], W1bd[0:4, :512],
                                     start=True, stop=True, tile_position=(0, 0))
                    nc.tensor.matmul(zAo[:], aTc[32:36, sl], W1bd[32:36, :512],
                                     start=True, stop=True, tile_position=(32, 0))
                    nc.tensor.matmul(zBe[:], aTc[64:68, sl], W1bd[64:68, 512:],
                                     start=True, stop=True, tile_position=(64, 0))
                    nc.tensor.matmul(zBo[:], aTc[96:100, sl], W1bd[96:100, 512:],
                                     start=True, stop=True, tile_position=(96, 0))
                    nc.scalar.activation(mp[:, 0, :512], zAe[:], Relu, scale=MS)
                    nc.vector.tensor_scalar(mp[:, 0, 512:], zBe[:], MS, 0.0,
                                            op0=Mult, op1=Max)
                    nc.scalar.activation(mp[:, 1, :512], zAo[:], Relu, scale=MS)
                    nc.vector.tensor_scalar(mp[:, 1, 512:], zBo[:], MS, 0.0,
                                            op0=Mult, op1=Max)
                else:
                    mp = tails[state["tail"]]
                    state["tail"] ^= 1
                    zA = zp.tile([P, 512], f32, tag="z")
                    zB = zp.tile([P, 512], f32, tag="z")
                    nc.tensor.matmul(zA[:], aTc[0:4, sl], W1bd[0:4, :512],
                                     start=True, stop=True, tile_position=(0, 0))
                    nc.tensor.matmul(zB[:], aTc[64:68, sl], W1bd[64:68, 512:],
                                     start=True, stop=True, tile_position=(64, 0))
                    nc.scalar.activation(mp[:, 0, :512], zA[:], Relu, scale=MS)
                    nc.scalar.activation(mp[:, 0, 512:], zB[:], Relu, scale=MS)
                msg_of[blk] = mp

            def emit_block_if(target):
                if state["b"] < min(target, NBLK):
                    emit_block()

            # prologue: window 0's blocks
            stage_chunk(0)
            while state["b"] < int(base_blk[1]):
                emit_block()

            for w in range(nw):
                Tt = sT.tile([P, G], bf16, tag="Tt")
                nc.sync.dma_start(Tt[:], T_d[w])
                nb = int(nblk_w[w])
                b0 = int(base_blk[w])
                target = int(base_blk[min(w + 2, nw)])

                # dual-form fp8 DoubleRow aggregation:
                # g2T[f,dst] += sum_pair msg[e,f].T @ S[e,dst]
                g2T = sg2T.tile([P, 8, P], fp8, tag="g2T")
                for p4 in range(4):
                    gA = gp.tile([P, 512], f32, tag="g")
                    gB = gp.tile([P, 512], f32, tag="g")
                    jA, jB = 2 * p4, 2 * p4 + 1
                    for b in range(nb):
                        blk = b0 + b
                        nc.tensor.matmul(
                            gA[:, :P], msg_of[blk][:, :, jA * P : (jA + 1) * P],
                            Ss_of[blk][:], start=(b == 0), stop=(b == nb - 1),
                            perf_mode=DR,
                        )
                        nc.tensor.matmul(
                            gB[:, :P], msg_of[blk][:, :, jB * P : (jB + 1) * P],
                            Ss_of[blk][:], start=(b == 0), stop=(b == nb - 1),
                            perf_mode=DR,
                        )
                    nc.scalar.activation(g2T[:, jA], gA[:, :P], Copy,
                                         scale=GS / MS)
                    nc.vector.tensor_scalar_mul(g2T[:, jB], gB[:, :P], GS / MS)
                    if p4 < 3:
                        emit_block_if(target)   # cover gp drain w/ L1 stream

                # h2 = relu((g2*GS @ W2*16)/128 + b2); fp8 DoubleRow pairs
                hps = []
                for half in range(2):
                    lo = half * 512
                    h2p = hp.tile([P, 512], f32, tag="h")
                    for j2 in range(4):
                        nc.tensor.matmul(
                            h2p[:], g2T[:, 2 * j2 : 2 * j2 + 2, :],
                            W2s[:, 2 * j2 : 2 * j2 + 2, lo : lo + 512],
                            start=(j2 == 0), stop=False, perf_mode=DR,
                        )
                    nc.tensor.matmul(
                        h2p[:], ones1[:1, :], b2s[:1, lo : lo + 512],
                        start=False, stop=True,
                    )
                    hps.append(h2p)
                    if half == 0:
                        emit_block_if(target)
                h2b = sh2.tile([P, H], bf16, tag="h2b")
                nc.scalar.activation(h2b[:, :512], hps[0][:], Relu,
                                     scale=1.0 / 128)
                nc.scalar.activation(h2b[:, 512:], hps[1][:], Relu,
                                     scale=1.0 / 128)
                emit_block_if(target)
                for half in range(2):
                    lo = half * 512
                    cp = hp.tile([P, 512], f32, tag="h")
                    nc.tensor.matmul(
                        cp[:], Tt[:], h2b[:, lo : lo + 512], start=True, stop=True
                    )
                    nc.vector.tensor_add(
                        pg3s[:, lo : lo + 512], pg3s[:, lo : lo + 512], cp[:]
                    )
                for b in range(nb):
                    msg_of.pop(b0 + b, None)
                    Ss_of.pop(b0 + b, None)

            nc.sync.dma_start(out_d[:], pg3s[:])

    nc.finalize()
    return nc


def kernel(x, W1, b1, W2, b2, W3, b3, Wlin, blin, edge_index, batch, num_graphs):
    import ml_dtypes
    from concourse.bass_utils import run_bass_kernel_spmd

    bf16 = ml_dtypes.bfloat16
    fp8 = ml_dtypes.float8_e4m3
    x = np.asarray(x, dtype=np.float32)
    W1 = np.asarray(W1, dtype=np.float32)
    b1 = np.asarray(b1, dtype=np.float32)
    W2 = np.asarray(W2, dtype=np.float32)
    b2 = np.asarray(b2, dtype=np.float32)
    W3 = np.asarray(W3, dtype=np.float32)
    b3 = np.asarray(b3, dtype=np.float32)
    Wlin = np.asarray(Wlin, dtype=np.float32)
    blin = np.asarray(blin, dtype=np.float32)

    (aT4, S2, Tpad, cnt, nblk_w, blk_kind, NBLK, base_blk) = _host_prep(
        x, edge_index, batch
    )

    nc = _build_device_program(NBLK, nblk_w, blk_kind, base_blk)

    W1b = np.zeros((4, H), dtype=np.float32)
    W1b[:2] = W1
    W1b[2] = b1
    W1b = W1b.astype(bf16)
    W2r = np.ascontiguousarray((W2 * 16.0).reshape(8, P, H)).astype(fp8)
    b2r = (b2 * 128.0).reshape(1, H).astype(bf16)

    in_maps = [
        {
            "aT": np.ascontiguousarray(aT4[c]),
            "S": np.ascontiguousarray(S2[c]),
            "T": np.ascontiguousarray(Tpad[c].reshape(NW, P, G)),
            "W1b": W1b,
            "W2": W2r,
            "b2": b2r,
        }
        for c in range(N_CORES)
    ]
    res = run_bass_kernel_spmd(nc, in_maps, core_ids=list(range(N_CORES)))
    global LAST_RESULTS
    LAST_RESULTS = res
    pg3 = np.zeros((G, H), dtype=np.float64)
    for r in res.results:
        pg3 += r["pg3"].astype(np.float64)
    pg3 = pg3.astype(np.float32)

    pooled = (pg3 @ W3 + cnt[:, None] * b3[None, :]) / np.maximum(cnt, 1.0)[:, None]
    out = pooled @ Wlin + blin[None, :]
    return out.astype(np.float32)


# revision 31
# speedup vs baseline: 1.3575x; 1.3575x over previous
"""GCN (3-layer, PyG-style) forward on 8 Trainium2 NeuronCores.

Math restructuring
------------------
reference:
  h1 = relu(Anorm @ x @ W1 + b1)          (Anorm includes self loops + sym norm)
  h2 = relu(Anorm @ h1 @ W2 + b2)
  h3 = Anorm @ h2 @ W3 + b3
  out = segment_mean(h3, batch) @ Wlin + blin

Because GCNConv aggregation and the weight matmul commute, and pooling is
linear, this is equivalent to:
  agg1 = Anorm @ x                        # [N,2]  (tiny -> host)
  msg_e = relu(norm_e * (agg1[src_e] @ W1 + b1))    # per-edge (norm>0 commutes
                                                    #  through relu)
  g2   = scatter-sum msg to dst           # exact one-hot matmul on device
  h2   = relu(g2 @ W2 + b2)               # dense matmul on device
  pg3[g] = sum_n T[n,g] * h2[n]           # T[n,g] = sum of norm over n's
                                          #  out-edges into graph g
  out  = ((pg3 @ W3 + cnt*b3)/max(cnt,1)) @ Wlin + blin   # [128,1024] -> host

Sharding: nodes are LPT bin-packed into 8 cores x 98 windows of 128 slots so
that each (core, window) bin holds ~638 incident edges (load-balanced).  Every
core runs the same program (SPMD) on its own edge arrays, padded to identical
tile counts.  Per-core output is a partial pg3 [128,1024]; the host sums them
(the "all-reduce").

Device-side structure:
 - L1 (K=4, bf16) matmuls are 4-way row-group packed via tile_position: aT and
   W1b are duplicated at SBUF base partitions {0,32,64,96} so the four matmuls
   of an edge-tile pair run concurrently in separate 32-row strips
 - messages are stored fp8e4m3 (x16); the aggregation runs in "dual form"
   (g2T[f,dst] += msg[e,f].T @ S[e,dst]) as fp8 DoubleRow matmuls that pack
   two edge tiles per instruction; S is an exact one-hot so a window's odd
   tail tile is paired with an all-zero S column (masking a dummy slot)
 - h2 = relu(g2 @ W2 + b2) also runs as fp8 DoubleRow (g2T x8, W2 x16,
   rescaled 1/128 inside the relu); pg3 accumulation is bf16->fp32
 - L1 work for window w+1 is interleaved between the aggregation passes of
   window w, keeping every matmul's dependencies one window ahead (dense PE
   stream, HAM stays warm) and covering the PSUM drain latencies
"""

import numpy as np

LAST_RESULTS = None  # set by kernel() for test harness introspection

N_NODES = 100000
N_EDGES = 400000
G = 128
FIN = 2
H = 1024
N_CORES = 8
P = 128
NW = 98                      # windows per core (98*128 = 12544 >= 12500 slots)
NBINS = N_CORES * NW


def _lpt_pack(wgt):
    """Assign each node to one of 784 (core,window) bins, balancing total
    edge weight per bin with a <=128 nodes/bin cap.  Returns (bin_of, slot_of).
    """
    import heapq

    n = len(wgt)
    order = np.argsort(-wgt, kind="stable")
    heap = [(0, 0, b) for b in range(NBINS)]
    heapq.heapify(heap)
    bin_of = np.empty(n, dtype=np.int64)
    slot_of = np.empty(n, dtype=np.int64)
    w_arr = wgt.tolist()
    for idx in order.tolist():
        while True:
            load, count, b = heapq.heappop(heap)
            if count < P:
                break
        bin_of[idx] = b
        slot_of[idx] = count
        heapq.heappush(heap, (load + w_arr[idx], count + 1, b))
    return bin_of, slot_of


def _host_prep(x, edge_index, batch):
    """All O(E) index work in numpy; returns per-core device arrays."""
    import ml_dtypes

    bf16 = ml_dtypes.bfloat16
    fp8 = ml_dtypes.float8_e4m3
    x = np.asarray(x, dtype=np.float32)
    ei = np.asarray(edge_index).astype(np.int64)
    batch = np.asarray(batch).astype(np.int64)
    n = N_NODES

    loops = np.arange(n, dtype=np.int64)
    row = np.concatenate([ei[0], loops])
    col = np.concatenate([ei[1], loops])

    deg = np.bincount(col, minlength=n).astype(np.float64)
    dis = np.where(deg > 0, 1.0 / np.sqrt(np.maximum(deg, 1.0)), 0.0)
    norm = dis[row] * dis[col]                     # fp64

    # layer-1 aggregation (FIN=2) on host
    agg1 = np.empty((n, FIN), dtype=np.float64)
    for f in range(FIN):
        agg1[:, f] = np.bincount(
            col, weights=norm * x[row, f].astype(np.float64), minlength=n
        )

    # ---- node -> (core, window, slot) via LPT packing on indegree+1 ----
    wgt = np.bincount(col, minlength=n)            # includes the self loop
    bin_raw, slot_of = _lpt_pack(wgt)
    # deal bins to (core, window) so similar loads share a window
    loads = np.zeros(NBINS, dtype=np.int64)
    np.add.at(loads, bin_raw, wgt)
    deal = np.argsort(-loads, kind="stable")       # deal[k] = raw bin id
    bin_rank = np.empty(NBINS, dtype=np.int64)
    bin_rank[deal] = np.arange(NBINS)
    rank = bin_rank[bin_raw]                       # 0..783, sorted by load
    node_w = rank // N_CORES                       # window 0..97
    node_c = rank % N_CORES                        # core 0..7

    # ---- edges ordered by (dst core, dst window) ----
    e_rank = rank[col]
    order = np.argsort(e_rank, kind="stable")
    row_s, col_s = row[order], col[order]
    norm_s = norm[order]
    rank_s = e_rank[order]
    c_s = rank_s % N_CORES
    w_s = rank_s // N_CORES

    cnts = np.bincount(e_rank, minlength=NBINS)    # indexed by rank = w*8 + c
    cw_load = cnts.reshape(NW, N_CORES).T          # [core, window]
    T_w = ((cw_load.max(axis=0) + P - 1) // P).astype(np.int64)   # per window
    base_tile = np.concatenate([[0], np.cumsum(T_w)])
    TT = int(base_tile[-1])

    starts = np.concatenate([[0], np.cumsum(cnts)])
    idx_in_bin = np.arange(len(col_s)) - starts[rank_s]
    tile_g = base_tile[w_s] + idx_in_bin // P
    slot = tile_g * P + idx_in_bin % P

    # flat per-slot arrays (norm folded into aT; S is exact one-hot)
    aT = np.zeros((N_CORES, 4, TT * P), dtype=np.float32)
    Sfull = np.zeros((N_CORES, TT * P, P), dtype=bf16)
    aT[c_s, 0, slot] = (agg1[row_s, 0] * norm_s).astype(np.float32)
    aT[c_s, 1, slot] = (agg1[row_s, 1] * norm_s).astype(np.float32)
    aT[c_s, 2, slot] = norm_s.astype(np.float32)
    Sfull[c_s, slot, slot_of[col_s]] = bf16(1.0)
    aTt = aT.reshape(N_CORES, 4, TT, P)
    Sfull = Sfull.reshape(N_CORES, TT, P, P)

    # ---- window-local tile pairing into "blocks" (2 tiles per block) ----
    # block kinds: 1 = full pair, 0 = tail (second slot S-masked to zero)
    nblk_w = ((T_w + 1) // 2).astype(np.int64)     # blocks per window
    base_blk = np.concatenate([[0], np.cumsum(nblk_w)])
    NBLK = int(base_blk[-1])

    aT4 = np.zeros((N_CORES, 16, NBLK * P), dtype=np.float32)
    S2 = np.zeros((N_CORES, NBLK, P, 2, P), dtype=bf16)
    blk_kind = np.zeros(NBLK, dtype=np.int64)
    for w in range(NW):
        nt = int(T_w[w])
        for b in range(int(nblk_w[w])):
            blk = int(base_blk[w]) + b
            t0 = int(base_tile[w]) + 2 * b
            csl = slice(blk * P, (blk + 1) * P)
            aT4[:, 0:4, csl] = aTt[:, :, t0]
            aT4[:, 8:12, csl] = aTt[:, :, t0]
            S2[:, blk, :, 0, :] = Sfull[:, t0]
            if 2 * b + 1 < nt:
                blk_kind[blk] = 1
                aT4[:, 4:8, csl] = aTt[:, :, t0 + 1]
                aT4[:, 12:16, csl] = aTt[:, :, t0 + 1]
                S2[:, blk, :, 1, :] = Sfull[:, t0 + 1]

    # ---- L3: T matrix rows permuted to node home slots ----
    gcol = batch[col]                              # graph of each edge's dst
    Tmat = np.bincount(
        row * G + gcol, weights=norm, minlength=n * G
    ).astype(np.float32).reshape(n, G)
    Tpad = np.zeros((N_CORES, NW * P, G), dtype=bf16)
    Tpad[node_c, node_w * P + slot_of] = Tmat.astype(bf16)

    cnt = np.bincount(batch, minlength=G).astype(np.float32)
    return (aT4.astype(bf16), S2, Tpad, cnt, nblk_w, blk_kind, NBLK, base_blk)


def _build_device_program(NBLK, nblk_w, blk_kind, base_blk, nw=NW):
    import concourse.mybir as mybir
    import concourse.tile as tile
    from concourse import bacc

    f32 = mybir.dt.float32
    bf16 = mybir.dt.bfloat16
    fp8 = mybir.dt.float8e4
    DR = mybir.MatmulPerfMode.DoubleRow
    nc = bacc.Bacc(None, target_bir_lowering=False, debug=False)

    aT_d = nc.dram_tensor("aT", [16, NBLK * P], bf16, kind="ExternalInput")
    S_d = nc.dram_tensor("S", [NBLK, P, 2, P], bf16, kind="ExternalInput")
    T_d = nc.dram_tensor("T", [NW, P, G], bf16, kind="ExternalInput")
    W1b_d = nc.dram_tensor("W1b", [4, H], bf16, kind="ExternalInput")
    W2_d = nc.dram_tensor("W2", [8, P, H], fp8, kind="ExternalInput")
    b2_d = nc.dram_tensor("b2", [1, H], bf16, kind="ExternalInput")
    out_d = nc.dram_tensor("pg3", [G, H], f32, kind="ExternalOutput")

    CH = 16                      # aT blocks per staged chunk
    n_chunks = (NBLK + CH - 1) // CH
    GS = 8.0                     # g2T fp8 scale

    with tile.TileContext(nc) as tc:
        with (
            tc.tile_pool(name="const", bufs=1) as cst,
            tc.tile_pool(name="sa", bufs=2) as sa,
            tc.tile_pool(name="sS", bufs=10) as sS,
            tc.tile_pool(name="smsg", bufs=8) as smsg,
            tc.tile_pool(name="sg2T", bufs=2) as sg2T,
            tc.tile_pool(name="sh2", bufs=2) as sh2,
            tc.tile_pool(name="sT", bufs=2) as sT,
            tc.tile_pool(name="zp", bufs=4, space="PSUM") as zp,
            tc.tile_pool(name="gp", bufs=2, space="PSUM") as gp,
            tc.tile_pool(name="hp", bufs=2, space="PSUM") as hp,
        ):
            Relu = mybir.ActivationFunctionType.Relu
            Copy = mybir.ActivationFunctionType.Copy
            Mult = mybir.AluOpType.mult
            Max = mybir.AluOpType.max

            # W1b duplicated at base partitions 0/32/64/96
            W1bd = cst.tile([100, H], bf16, tag="W1bd")
            for g4 in range(4):
                nc.sync.dma_start(W1bd[g4 * 32 : g4 * 32 + 4, :], W1b_d[:])
            W2s = cst.tile([P, 8, H], fp8, tag="W2s")
            nc.sync.dma_start(W2s[:], W2_d[:].rearrange("c p f -> p c f"))
            b2s = cst.tile([1, H], bf16, tag="b2s")
            nc.sync.dma_start(b2s[:], b2_d[:])
            ones1 = cst.tile([1, P], bf16, tag="ones1")
            nc.vector.memset(ones1[:], 1.0)
            pg3s = cst.tile([G, H], f32, tag="pg3s")
            nc.vector.memset(pg3s[:], 0.0)

            chunks = {}          # chunk idx -> staged aT tile
            msg_of = {}          # block -> msg pair tile [P, 2, H]
            Ss_of = {}           # block -> one-hot S pair tile [P, 2, P]

            def stage_chunk(ci):
                if ci >= n_chunks or ci in chunks:
                    return
                t_ = sa.tile([100, CH * P], bf16, tag="aTc")
                lo = ci * CH * P
                hi = min((ci + 1) * CH * P, NBLK * P)
                for g4 in range(4):
                    nc.sync.dma_start(
                        t_[g4 * 32 : g4 * 32 + 4, : hi - lo],
                        aT_d[g4 * 4 : g4 * 4 + 4, lo:hi],
                    )
                chunks[ci] = t_

            state = {"b": 0, "tail": 0}

            def emit_block():
                blk = state["b"]
                if blk >= NBLK:
                    return
                state["b"] = blk + 1
                ci, off = blk // CH, (blk % CH) * P
                if blk % CH == 0:
                    stage_chunk(ci + 1)
                aTc = chunks[ci]
                full = blk_kind[blk] == 1
                Ss = sS.tile([P, 2, P], bf16, tag="Ss")
                nc.sync.dma_start(Ss[:], S_d[blk])
                Ss_of[blk] = Ss
                sl = slice(off, off + P)
                mp = smsg.tile([P, 2, H], bf16, tag="msg")
                if full:
                    zAe = zp.tile([P, 512], f32, tag="z")
                    zAo = zp.tile([P, 512], f32, tag="z")
                    zBe = zp.tile([P, 512], f32, tag="z")
                    zBo = zp.tile([P, 512], f32, tag="z")
                    nc.tensor.matmul(zAe[:], aTc[0:4, sl], W1bd[0:4, :512],
                                     start=True, stop=True, tile_position=(0, 0))
                    nc.tensor.matmul(zAo[:], aTc[32:36, sl], W1bd[32:36, :512],
                                     start=True, stop=True, tile_position=(32, 0))
                    nc.tensor.matmul(zBe[:], aTc[64:68, sl], W1bd[64:68, 512:],
                                     start=True, stop=True, tile_position=(64, 0))
                    nc.tensor.matmul(zBo[:], aTc[96:100, sl], W1bd[96:100, 512:],
                                     start=True, stop=True, tile_position=(96, 0))
                    nc.scalar.activation(mp[:, 0, :512], zAe[:], Relu)
                    nc.vector.tensor_scalar_max(mp[:, 0, 512:], zBe[:], 0.0)
                    nc.scalar.activation(mp[:, 1, :512], zAo[:], Relu)
                    nc.vector.tensor_scalar_max(mp[:, 1, 512:], zBo[:], 0.0)
                else:
                    zA = zp.tile([P, 512], f32, tag="z")
                    zB = zp.tile([P, 512], f32, tag="z")
                    nc.tensor.matmul(zA[:], aTc[0:4, sl], W1bd[0:4, :512],
                                     start=True, stop=True, tile_position=(0, 0))
                    nc.tensor.matmul(zB[:], aTc[64:68, sl], W1bd[64:68, 512:],
                                     start=True, stop=True, tile_position=(64, 0))
                    nc.scalar.activation(mp[:, 0, :512], zA[:], Relu)
                    nc.scalar.activation(mp[:, 0, 512:], zB[:], Relu)
                msg_of[blk] = mp

            def emit_block_if(target):
                if state["b"] < min(target, NBLK):
                    emit_block()

            # prologue: window 0's blocks
            stage_chunk(0)
            while state["b"] < int(base_blk[1]):
                emit_block()

            for w in range(nw):
                Tt = sT.tile([P, G], bf16, tag="Tt")
                nc.sync.dma_start(Tt[:], T_d[w])
                nb = int(nblk_w[w])
                b0 = int(base_blk[w])
                target = int(base_blk[min(w + 2, nw)])

                # dual-form bf16 aggregation: g2T[f,dst] += msg[e,f].T @ S[e,dst]
                # (per tile slot; a tail block's unused slot 1 is skipped)
                slots = []
                for b in range(nb):
                    ns = 2 if blk_kind[b0 + b] == 1 else 1
                    slots += [(b0 + b, s_) for s_ in range(ns)]
                g2T = sg2T.tile([P, 8, P], fp8, tag="g2T")
                for p4 in range(4):
                    gA = gp.tile([P, 512], f32, tag="g")
                    gB = gp.tile([P, 512], f32, tag="g")
                    jA, jB = 2 * p4, 2 * p4 + 1
                    for si, (blk, s_) in enumerate(slots):
                        st_, sp_ = si == 0, si == len(slots) - 1
                        nc.tensor.matmul(
                            gA[:, :P], msg_of[blk][:, s_, jA * P : (jA + 1) * P],
                            Ss_of[blk][:, s_, :], start=st_, stop=sp_,
                        )
                        nc.tensor.matmul(
                            gB[:, :P], msg_of[blk][:, s_, jB * P : (jB + 1) * P],
                            Ss_of[blk][:, s_, :], start=st_, stop=sp_,
                        )
                    nc.scalar.activation(g2T[:, jA], gA[:, :P], Copy, scale=GS)
                    nc.vector.tensor_scalar_mul(g2T[:, jB], gB[:, :P], GS)
                    if p4 < 3:
                        emit_block_if(target)   # cover gp drain w/ L1 stream

                # h2 = relu((g2*GS @ W2*16)/128 + b2); fp8 DoubleRow pairs
                hps = []
                for half in range(2):
                    lo = half * 512
                    h2p = hp.tile([P, 512], f32, tag="h")
                    for j2 in range(4):
                        nc.tensor.matmul(
                            h2p[:], g2T[:, 2 * j2 : 2 * j2 + 2, :],
                            W2s[:, 2 * j2 : 2 * j2 + 2, lo : lo + 512],
                            start=(j2 == 0), stop=False, perf_mode=DR,
                        )
                    nc.tensor.matmul(
                        h2p[:], ones1[:1, :], b2s[:1, lo : lo + 512],
                        start=False, stop=True,
                    )
                    hps.append(h2p)
                    if half == 0:
                        emit_block_if(target)
                h2b = sh2.tile([P, H], bf16, tag="h2b")
                nc.scalar.activation(h2b[:, :512], hps[0][:], Relu,
                                     scale=1.0 / 128)
                nc.scalar.activation(h2b[:, 512:], hps[1][:], Relu,
                                     scale=1.0 / 128)
                emit_block_if(target)
                for half in range(2):
                    lo = half * 512
                    cp = hp.tile([P, 512], f32, tag="h")
                    nc.tensor.matmul(
                        cp[:], Tt[:], h2b[:, lo : lo + 512], start=True, stop=True
                    )
                    nc.vector.tensor_add(
                        pg3s[:, lo : lo + 512], pg3s[:, lo : lo + 512], cp[:]
                    )
                for b in range(nb):
                    msg_of.pop(b0 + b, None)
                    Ss_of.pop(b0 + b, None)

            nc.sync.dma_start(out_d[:], pg3s[:])

    nc.finalize()
    return nc


def kernel(x, W1, b1, W2, b2, W3, b3, Wlin, blin, edge_index, batch, num_graphs):
    import ml_dtypes
    from concourse.bass_utils import run_bass_kernel_spmd

    bf16 = ml_dtypes.bfloat16
    fp8 = ml_dtypes.float8_e4m3
    x = np.asarray(x, dtype=np.float32)
    W1 = np.asarray(W1, dtype=np.float32)
    b1 = np.asarray(b1, dtype=np.float32)
    W2 = np.asarray(W2, dtype=np.float32)
    b2 = np.asarray(b2, dtype=np.float32)
    W3 = np.asarray(W3, dtype=np.float32)
    b3 = np.asarray(b3, dtype=np.float32)
    Wlin = np.asarray(Wlin, dtype=np.float32)
    blin = np.asarray(blin, dtype=np.float32)

    (aT4, S2, Tpad, cnt, nblk_w, blk_kind, NBLK, base_blk) = _host_prep(
        x, edge_index, batch
    )

    nc = _build_device_program(NBLK, nblk_w, blk_kind, base_blk)

    W1b = np.zeros((4, H), dtype=np.float32)
    W1b[:2] = W1
    W1b[2] = b1
    W1b = W1b.astype(bf16)
    W2r = np.ascontiguousarray((W2 * 16.0).reshape(8, P, H)).astype(fp8)
    b2r = (b2 * 128.0).reshape(1, H).astype(bf16)

    in_maps = [
        {
            "aT": np.ascontiguousarray(aT4[c]),
            "S": np.ascontiguousarray(S2[c]),
            "T": np.ascontiguousarray(Tpad[c].reshape(NW, P, G)),
            "W1b": W1b,
            "W2": W2r,
            "b2": b2r,
        }
        for c in range(N_CORES)
    ]
    res = run_bass_kernel_spmd(nc, in_maps, core_ids=list(range(N_CORES)))
    global LAST_RESULTS
    LAST_RESULTS = res
    pg3 = np.zeros((G, H), dtype=np.float64)
    for r in res.results:
        pg3 += r["pg3"].astype(np.float64)
    pg3 = pg3.astype(np.float32)

    pooled = (pg3 @ W3 + cnt[:, None] * b3[None, :]) / np.maximum(cnt, 1.0)[:, None]
    out = pooled @ Wlin + blin[None, :]
    return out.astype(np.float32)


# revision 33
# speedup vs baseline: 1.5608x; 1.1498x over previous
"""GCN (3-layer, PyG-style) forward on 8 Trainium2 NeuronCores.

Math restructuring
------------------
reference:
  h1 = relu(Anorm @ x @ W1 + b1)          (Anorm includes self loops + sym norm)
  h2 = relu(Anorm @ h1 @ W2 + b2)
  h3 = Anorm @ h2 @ W3 + b3
  out = segment_mean(h3, batch) @ Wlin + blin

Because GCNConv aggregation and the weight matmul commute, and pooling is
linear, this is equivalent to:
  agg1 = Anorm @ x                        # [N,2]  (tiny -> host)
  msg_e = relu(norm_e * (agg1[src_e] @ W1 + b1))    # per-edge (norm>0 commutes
                                                    #  through relu)
  g2   = scatter-sum msg to dst           # exact one-hot matmul on device
  h2   = relu(g2 @ W2 + b2)               # dense matmul on device
  pg3[g] = sum_n T[n,g] * h2[n]           # T[n,g] = sum of norm over n's
                                          #  out-edges into graph g
  out  = ((pg3 @ W3 + cnt*b3)/max(cnt,1)) @ Wlin + blin   # [128,1024] -> host

Sharding: nodes are LPT bin-packed into 8 cores x 98 windows of 128 slots so
that each (core, window) bin holds ~638 incident edges (load-balanced).  Every
core runs the same program (SPMD) on its own edge arrays, padded to identical
tile counts.  Per-core output is a partial pg3 [128,1024]; the host sums them
(the "all-reduce").

Device-side structure:
 - L1 (K=4, bf16) matmuls are 4-way row-group packed via tile_position: aT and
   W1b are duplicated at SBUF base partitions {0,32,64,96} so the four matmuls
   of an edge-tile pair run concurrently in separate 32-row strips
 - messages are stored fp8e4m3 (x16); the aggregation runs in "dual form"
   (g2T[f,dst] += msg[e,f].T @ S[e,dst]) as fp8 DoubleRow matmuls that pack
   two edge tiles per instruction; S is an exact one-hot so a window's odd
   tail tile is paired with an all-zero S column (masking a dummy slot)
 - h2 = relu(g2 @ W2 + b2) also runs as fp8 DoubleRow (g2T x8, W2 x16,
   rescaled 1/128 inside the relu); pg3 accumulation is bf16->fp32
 - L1 work for window w+1 is interleaved between the aggregation passes of
   window w, keeping every matmul's dependencies one window ahead (dense PE
   stream, HAM stays warm) and covering the PSUM drain latencies
"""

import numpy as np

LAST_RESULTS = None  # set by kernel() for test harness introspection

N_NODES = 100000
N_EDGES = 400000
G = 128
FIN = 2
H = 1024
N_CORES = 8
P = 128
NW = 98                      # windows per core (98*128 = 12544 >= 12500 slots)
NBINS = N_CORES * NW


def _lpt_pack(wgt):
    """Assign each node to one of 784 (core,window) bins, balancing total
    edge weight per bin with a <=128 nodes/bin cap.  Returns (bin_of, slot_of).
    """
    import heapq

    n = len(wgt)
    order = np.argsort(-wgt, kind="stable")
    heap = [(0, 0, b) for b in range(NBINS)]
    heapq.heapify(heap)
    bin_of = np.empty(n, dtype=np.int64)
    slot_of = np.empty(n, dtype=np.int64)
    w_arr = wgt.tolist()
    for idx in order.tolist():
        while True:
            load, count, b = heapq.heappop(heap)
            if count < P:
                break
        bin_of[idx] = b
        slot_of[idx] = count
        heapq.heappush(heap, (load + w_arr[idx], count + 1, b))
    return bin_of, slot_of


def _host_prep(x, edge_index, batch):
    """All O(E) index work in numpy; returns per-core device arrays."""
    import ml_dtypes

    bf16 = ml_dtypes.bfloat16
    fp8 = ml_dtypes.float8_e4m3
    x = np.asarray(x, dtype=np.float32)
    ei = np.asarray(edge_index).astype(np.int64)
    batch = np.asarray(batch).astype(np.int64)
    n = N_NODES

    loops = np.arange(n, dtype=np.int64)
    row = np.concatenate([ei[0], loops])
    col = np.concatenate([ei[1], loops])

    deg = np.bincount(col, minlength=n).astype(np.float64)
    dis = np.where(deg > 0, 1.0 / np.sqrt(np.maximum(deg, 1.0)), 0.0)
    norm = dis[row] * dis[col]                     # fp64

    # layer-1 aggregation (FIN=2) on host
    agg1 = np.empty((n, FIN), dtype=np.float64)
    for f in range(FIN):
        agg1[:, f] = np.bincount(
            col, weights=norm * x[row, f].astype(np.float64), minlength=n
        )

    # ---- node -> (core, window, slot) via LPT packing on indegree+1 ----
    wgt = np.bincount(col, minlength=n)            # includes the self loop
    bin_raw, slot_of = _lpt_pack(wgt)
    # deal bins to (core, window) so similar loads share a window
    loads = np.zeros(NBINS, dtype=np.int64)
    np.add.at(loads, bin_raw, wgt)
    deal = np.argsort(-loads, kind="stable")       # deal[k] = raw bin id
    bin_rank = np.empty(NBINS, dtype=np.int64)
    bin_rank[deal] = np.arange(NBINS)
    rank = bin_rank[bin_raw]                       # 0..783, sorted by load
    node_w = rank // N_CORES                       # window 0..97
    node_c = rank % N_CORES                        # core 0..7

    # ---- edges ordered by (dst core, dst window) ----
    e_rank = rank[col]
    order = np.argsort(e_rank, kind="stable")
    row_s, col_s = row[order], col[order]
    norm_s = norm[order]
    rank_s = e_rank[order]
    c_s = rank_s % N_CORES
    w_s = rank_s // N_CORES

    cnts = np.bincount(e_rank, minlength=NBINS)    # indexed by rank = w*8 + c
    cw_load = cnts.reshape(NW, N_CORES).T          # [core, window]
    T_w = ((cw_load.max(axis=0) + P - 1) // P).astype(np.int64)   # per window
    base_tile = np.concatenate([[0], np.cumsum(T_w)])
    TT = int(base_tile[-1])

    starts = np.concatenate([[0], np.cumsum(cnts)])
    idx_in_bin = np.arange(len(col_s)) - starts[rank_s]
    tile_g = base_tile[w_s] + idx_in_bin // P
    slot = tile_g * P + idx_in_bin % P

    # flat per-slot arrays (norm folded into aT; S is exact one-hot)
    aT = np.zeros((N_CORES, 4, TT * P), dtype=np.float32)
    Sfull = np.zeros((N_CORES, TT * P, P), dtype=bf16)
    aT[c_s, 0, slot] = (agg1[row_s, 0] * norm_s).astype(np.float32)
    aT[c_s, 1, slot] = (agg1[row_s, 1] * norm_s).astype(np.float32)
    aT[c_s, 2, slot] = norm_s.astype(np.float32)
    Sfull[c_s, slot, slot_of[col_s]] = bf16(1.0)
    aTt = aT.reshape(N_CORES, 4, TT, P)
    Sfull = Sfull.reshape(N_CORES, TT, P, P)

    # ---- window-local tile pairing into "blocks" (2 tiles per block) ----
    # block kinds: 1 = full pair, 0 = tail (second slot S-masked to zero)
    nblk_w = ((T_w + 1) // 2).astype(np.int64)     # blocks per window
    base_blk = np.concatenate([[0], np.cumsum(nblk_w)])
    NBLK = int(base_blk[-1])

    aT4 = np.zeros((N_CORES, 16, NBLK * P), dtype=np.float32)
    S2 = np.zeros((N_CORES, NBLK, P, 2, P), dtype=bf16)
    blk_kind = np.zeros(NBLK, dtype=np.int64)
    for w in range(NW):
        nt = int(T_w[w])
        for b in range(int(nblk_w[w])):
            blk = int(base_blk[w]) + b
            t0 = int(base_tile[w]) + 2 * b
            csl = slice(blk * P, (blk + 1) * P)
            aT4[:, 0:4, csl] = aTt[:, :, t0]
            aT4[:, 8:12, csl] = aTt[:, :, t0]
            S2[:, blk, :, 0, :] = Sfull[:, t0]
            if 2 * b + 1 < nt:
                blk_kind[blk] = 1
                aT4[:, 4:8, csl] = aTt[:, :, t0 + 1]
                aT4[:, 12:16, csl] = aTt[:, :, t0 + 1]
                S2[:, blk, :, 1, :] = Sfull[:, t0 + 1]

    # ---- L3: T matrix rows permuted to node home slots ----
    gcol = batch[col]                              # graph of each edge's dst
    Tmat = np.bincount(
        row * G + gcol, weights=norm, minlength=n * G
    ).astype(np.float32).reshape(n, G)
    Tpad = np.zeros((N_CORES, NW * P, G), dtype=bf16)
    Tpad[node_c, node_w * P + slot_of] = Tmat.astype(bf16)

    cnt = np.bincount(batch, minlength=G).astype(np.float32)
    return (aT4.astype(bf16), S2, Tpad, cnt, nblk_w, blk_kind, NBLK, base_blk)


def _build_device_program(NBLK, nblk_w, blk_kind, base_blk, nw=NW):
    import concourse.mybir as mybir
    import concourse.tile as tile
    from concourse import bacc

    f32 = mybir.dt.float32
    bf16 = mybir.dt.bfloat16
    fp8 = mybir.dt.float8e4
    DR = mybir.MatmulPerfMode.DoubleRow
    nc = bacc.Bacc(None, target_bir_lowering=False, debug=False)

    aT_d = nc.dram_tensor("aT", [16, NBLK * P], bf16, kind="ExternalInput")
    S_d = nc.dram_tensor("S", [NBLK, P, 2, P], bf16, kind="ExternalInput")
    T_d = nc.dram_tensor("T", [NW, P, G], bf16, kind="ExternalInput")
    W1b_d = nc.dram_tensor("W1b", [4, H], bf16, kind="ExternalInput")
    W2_d = nc.dram_tensor("W2", [8, P, H], fp8, kind="ExternalInput")
    b2_d = nc.dram_tensor("b2", [1, H], bf16, kind="ExternalInput")
    out_d = nc.dram_tensor("pg3", [G, H], f32, kind="ExternalOutput")

    CH = 16                      # aT blocks per staged chunk
    n_chunks = (NBLK + CH - 1) // CH
    GS = 8.0                     # g2T fp8 scale

    with tile.TileContext(nc) as tc:
        with (
            tc.tile_pool(name="const", bufs=1) as cst,
            tc.tile_pool(name="sa", bufs=2) as sa,
            tc.tile_pool(name="sS", bufs=10) as sS,
            tc.tile_pool(name="smsg", bufs=8) as smsg,
            tc.tile_pool(name="sg2T", bufs=2) as sg2T,
            tc.tile_pool(name="sh2", bufs=2) as sh2,
            tc.tile_pool(name="sT", bufs=2) as sT,
            tc.tile_pool(name="zp", bufs=4, space="PSUM") as zp,
            tc.tile_pool(name="gp", bufs=2, space="PSUM") as gp,
            tc.tile_pool(name="hp", bufs=2, space="PSUM") as hp,
        ):
            Relu = mybir.ActivationFunctionType.Relu
            Copy = mybir.ActivationFunctionType.Copy
            Mult = mybir.AluOpType.mult
            Max = mybir.AluOpType.max

            # W1b duplicated at base partitions 0/32/64/96
            W1bd = cst.tile([100, H], bf16, tag="W1bd")
            for g4 in range(4):
                nc.sync.dma_start(W1bd[g4 * 32 : g4 * 32 + 4, :], W1b_d[:])
            W2s = cst.tile([P, 8, H], fp8, tag="W2s")
            nc.sync.dma_start(W2s[:], W2_d[:].rearrange("c p f -> p c f"))
            b2s = cst.tile([1, H], bf16, tag="b2s")
            nc.sync.dma_start(b2s[:], b2_d[:])
            ones1 = cst.tile([1, P], bf16, tag="ones1")
            nc.vector.memset(ones1[:], 1.0)
            pg3s = cst.tile([G, H], f32, tag="pg3s")
            nc.vector.memset(pg3s[:], 0.0)

            chunks = {}          # chunk idx -> staged aT tile
            msg_of = {}          # block -> msg pair tile [P, 2, H]
            Ss_of = {}           # block -> one-hot S pair tile [P, 2, P]

            def stage_chunk(ci):
                if ci >= n_chunks or ci in chunks:
                    return
                t_ = sa.tile([100, CH * P], bf16, tag="aTc")
                lo = ci * CH * P
                hi = min((ci + 1) * CH * P, NBLK * P)
                for g4 in range(4):
                    nc.sync.dma_start(
                        t_[g4 * 32 : g4 * 32 + 4, : hi - lo],
                        aT_d[g4 * 4 : g4 * 4 + 4, lo:hi],
                    )
                chunks[ci] = t_

            state = {"b": 0, "tail": 0}

            def emit_block():
                blk = state["b"]
                if blk >= NBLK:
                    return
                state["b"] = blk + 1
                ci, off = blk // CH, (blk % CH) * P
                if blk % CH == 0:
                    stage_chunk(ci + 1)
                aTc = chunks[ci]
                full = blk_kind[blk] == 1
                Ss = sS.tile([P, 2, P], bf16, tag="Ss")
                nc.sync.dma_start(Ss[:], S_d[blk])
                Ss_of[blk] = Ss
                sl = slice(off, off + P)
                mp = smsg.tile([P, 2, H], bf16, tag="msg")
                if full:
                    zAe = zp.tile([P, 512], f32, tag="z")
                    zAo = zp.tile([P, 512], f32, tag="z")
                    zBe = zp.tile([P, 512], f32, tag="z")
                    zBo = zp.tile([P, 512], f32, tag="z")
                    nc.tensor.matmul(zAe[:], aTc[0:4, sl], W1bd[0:4, :512],
                                     start=True, stop=True, tile_position=(0, 0))
                    nc.tensor.matmul(zAo[:], aTc[32:36, sl], W1bd[32:36, :512],
                                     start=True, stop=True, tile_position=(32, 0))
                    nc.tensor.matmul(zBe[:], aTc[64:68, sl], W1bd[64:68, 512:],
                                     start=True, stop=True, tile_position=(64, 0))
                    nc.tensor.matmul(zBo[:], aTc[96:100, sl], W1bd[96:100, 512:],
                                     start=True, stop=True, tile_position=(96, 0))
                    nc.scalar.activation(mp[:, 0, :512], zAe[:], Relu)
                    nc.vector.tensor_scalar_max(mp[:, 0, 512:], zBe[:], 0.0)
                    nc.scalar.activation(mp[:, 1, :512], zAo[:], Relu)
                    nc.vector.tensor_scalar_max(mp[:, 1, 512:], zBo[:], 0.0)
                else:
                    zA = zp.tile([P, 512], f32, tag="z")
                    zB = zp.tile([P, 512], f32, tag="z")
                    nc.tensor.matmul(zA[:], aTc[0:4, sl], W1bd[0:4, :512],
                                     start=True, stop=True, tile_position=(0, 0))
                    nc.tensor.matmul(zB[:], aTc[64:68, sl], W1bd[64:68, 512:],
                                     start=True, stop=True, tile_position=(64, 0))
                    nc.scalar.activation(mp[:, 0, :512], zA[:], Relu)
                    nc.vector.tensor_scalar_max(mp[:, 0, 512:], zB[:], 0.0)
                msg_of[blk] = mp

            def emit_block_if(target):
                if state["b"] < min(target, NBLK):
                    emit_block()

            # prologue: window 0's blocks
            stage_chunk(0)
            while state["b"] < int(base_blk[1]):
                emit_block()

            for w in range(nw):
                Tt = sT.tile([P, G], bf16, tag="Tt")
                nc.sync.dma_start(Tt[:], T_d[w])
                nb = int(nblk_w[w])
                b0 = int(base_blk[w])
                target = int(base_blk[min(w + 2, nw)])

                # dual-form bf16 aggregation: g2T[f,dst] += msg[e,f].T @ S[e,dst]
                # (per tile slot; a tail block's unused slot 1 is skipped)
                slots = []
                for b in range(nb):
                    ns = 2 if blk_kind[b0 + b] == 1 else 1
                    slots += [(b0 + b, s_) for s_ in range(ns)]
                g2T = sg2T.tile([P, 8, P], fp8, tag="g2T")
                for p4 in range(4):
                    gA = gp.tile([P, 512], f32, tag="g")
                    gB = gp.tile([P, 512], f32, tag="g")
                    jA, jB = 2 * p4, 2 * p4 + 1
                    for si, (blk, s_) in enumerate(slots):
                        st_, sp_ = si == 0, si == len(slots) - 1
                        nc.tensor.matmul(
                            gA[:, :P], msg_of[blk][:, s_, jA * P : (jA + 1) * P],
                            Ss_of[blk][:, s_, :], start=st_, stop=sp_,
                        )
                        nc.tensor.matmul(
                            gB[:, :P], msg_of[blk][:, s_, jB * P : (jB + 1) * P],
                            Ss_of[blk][:, s_, :], start=st_, stop=sp_,
                        )
                    nc.scalar.activation(g2T[:, jA], gA[:, :P], Copy, scale=GS)
                    nc.vector.tensor_scalar_mul(g2T[:, jB], gB[:, :P], GS)
                    if p4 < 3:
                        emit_block_if(target)   # cover gp drain w/ L1 stream

                # h2 = relu((g2*GS @ W2*16)/128 + b2); fp8 DoubleRow pairs
                hps = []
                for half in range(2):
                    lo = half * 512
                    h2p = hp.tile([P, 512], f32, tag="h")
                    for j2 in range(4):
                        nc.tensor.matmul(
                            h2p[:], g2T[:, 2 * j2 : 2 * j2 + 2, :],
                            W2s[:, 2 * j2 : 2 * j2 + 2, lo : lo + 512],
                            start=(j2 == 0), stop=False, perf_mode=DR,
                        )
                    nc.tensor.matmul(
                        h2p[:], ones1[:1, :], b2s[:1, lo : lo + 512],
                        start=False, stop=True,
                    )
                    hps.append(h2p)
                    if half == 0:
                        emit_block_if(target)
                h2b = sh2.tile([P, H], bf16, tag="h2b")
                nc.scalar.activation(h2b[:, :512], hps[0][:], Relu,
                                     scale=1.0 / 128)
                nc.vector.tensor_scalar(
                    h2b[:, 512:], hps[1][:], 1.0 / 128, 0.0,
                    op0=Mult, op1=Max,
                )
                emit_block_if(target)
                for half in range(2):
                    lo = half * 512
                    cp = hp.tile([P, 512], f32, tag="h")
                    nc.tensor.matmul(
                        cp[:], Tt[:], h2b[:, lo : lo + 512], start=True, stop=True
                    )
                    nc.vector.tensor_add(
                        pg3s[:, lo : lo + 512], pg3s[:, lo : lo + 512], cp[:]
                    )
                for b in range(nb):
                    msg_of.pop(b0 + b, None)
                    Ss_of.pop(b0 + b, None)

            nc.sync.dma_start(out_d[:], pg3s[:])

    nc.finalize()
    return nc


def kernel(x, W1, b1, W2, b2, W3, b3, Wlin, blin, edge_index, batch, num_graphs):
    import ml_dtypes
    from concourse.bass_utils import run_bass_kernel_spmd

    bf16 = ml_dtypes.bfloat16
    fp8 = ml_dtypes.float8_e4m3
    x = np.asarray(x, dtype=np.float32)
    W1 = np.asarray(W1, dtype=np.float32)
    b1 = np.asarray(b1, dtype=np.float32)
    W2 = np.asarray(W2, dtype=np.float32)
    b2 = np.asarray(b2, dtype=np.float32)
    W3 = np.asarray(W3, dtype=np.float32)
    b3 = np.asarray(b3, dtype=np.float32)
    Wlin = np.asarray(Wlin, dtype=np.float32)
    blin = np.asarray(blin, dtype=np.float32)

    (aT4, S2, Tpad, cnt, nblk_w, blk_kind, NBLK, base_blk) = _host_prep(
        x, edge_index, batch
    )

    nc = _build_device_program(NBLK, nblk_w, blk_kind, base_blk)

    W1b = np.zeros((4, H), dtype=np.float32)
    W1b[:2] = W1
    W1b[2] = b1
    W1b = W1b.astype(bf16)
    W2r = np.ascontiguousarray((W2 * 16.0).reshape(8, P, H)).astype(fp8)
    b2r = (b2 * 128.0).reshape(1, H).astype(bf16)

    in_maps = [
        {
            "aT": np.ascontiguousarray(aT4[c]),
            "S": np.ascontiguousarray(S2[c]),
            "T": np.ascontiguousarray(Tpad[c].reshape(NW, P, G)),
            "W1b": W1b,
            "W2": W2r,
            "b2": b2r,
        }
        for c in range(N_CORES)
    ]
    res = run_bass_kernel_spmd(nc, in_maps, core_ids=list(range(N_CORES)))
    global LAST_RESULTS
    LAST_RESULTS = res
    pg3 = np.zeros((G, H), dtype=np.float64)
    for r in res.results:
        pg3 += r["pg3"].astype(np.float64)
    pg3 = pg3.astype(np.float32)

    pooled = (pg3 @ W3 + cnt[:, None] * b3[None, :]) / np.maximum(cnt, 1.0)[:, None]
    out = pooled @ Wlin + blin[None, :]
    return out.astype(np.float32)


# revision 41
# speedup vs baseline: 1.6986x; 1.0883x over previous
"""GCN (3-layer, PyG-style) forward on 8 Trainium2 NeuronCores.

Math restructuring
------------------
reference:
  h1 = relu(Anorm @ x @ W1 + b1)          (Anorm includes self loops + sym norm)
  h2 = relu(Anorm @ h1 @ W2 + b2)
  h3 = Anorm @ h2 @ W3 + b3
  out = segment_mean(h3, batch) @ Wlin + blin

Because GCNConv aggregation and the weight matmul commute, and pooling is
linear, this is equivalent to:
  agg1 = Anorm @ x                        # [N,2]  (tiny -> host)
  msg_e = relu(norm_e * (agg1[src_e] @ W1 + b1))    # per-edge (norm>0 commutes
                                                    #  through relu)
  g2   = scatter-sum msg to dst           # exact one-hot matmul on device
  h2   = relu(g2 @ W2 + b2)               # dense matmul on device
  pg3[g] = sum_n T[n,g] * h2[n]           # T[n,g] = sum of norm over n's
                                          #  out-edges into graph g
  out  = ((pg3 @ W3 + cnt*b3)/max(cnt,1)) @ Wlin + blin   # [128,1024] -> host

Sharding: nodes are LPT bin-packed into 8 cores x 98 windows of 128 slots so
that each (core, window) bin holds ~638 incident edges (load-balanced).  Every
core runs the same program (SPMD) on its own edge arrays, padded to identical
tile counts.  Per-core output is a partial pg3 [128,1024]; the host sums them
(the "all-reduce").

Device-side structure:
 - L1 (K=4, bf16) matmuls are 4-way row-group packed via tile_position: aT and
   W1b are duplicated at SBUF base partitions {0,32,64,96} so the four matmuls
   of an edge-tile pair run concurrently in separate 32-row strips
 - messages are stored fp8e4m3 (x16); the aggregation runs in "dual form"
   (g2T[f,dst] += msg[e,f].T @ S[e,dst]) as fp8 DoubleRow matmuls that pack
   two edge tiles per instruction; S is an exact one-hot so a window's odd
   tail tile is paired with an all-zero S column (masking a dummy slot)
 - h2 = relu(g2 @ W2 + b2) also runs as fp8 DoubleRow (g2T x8, W2 x16,
   rescaled 1/128 inside the relu); pg3 accumulation is bf16->fp32
 - L1 work for window w+1 is interleaved between the aggregation passes of
   window w, keeping every matmul's dependencies one window ahead (dense PE
   stream, HAM stays warm) and covering the PSUM drain latencies
"""

import numpy as np

LAST_RESULTS = None  # set by kernel() for test harness introspection

N_NODES = 100000
N_EDGES = 400000
G = 128
FIN = 2
H = 1024
N_CORES = 8
P = 128
NW = 98                      # windows per core (98*128 = 12544 >= 12500 slots)
NBINS = N_CORES * NW


def _lpt_pack(wgt):
    """Assign each node to one of 784 (core,window) bins, balancing total
    edge weight per bin with a <=128 nodes/bin cap.  Returns (bin_of, slot_of).
    """
    import heapq

    n = len(wgt)
    order = np.argsort(-wgt, kind="stable")
    heap = [(0, 0, b) for b in range(NBINS)]
    heapq.heapify(heap)
    bin_of = np.empty(n, dtype=np.int64)
    slot_of = np.empty(n, dtype=np.int64)
    w_arr = wgt.tolist()
    for idx in order.tolist():
        while True:
            load, count, b = heapq.heappop(heap)
            if count < P:
                break
        bin_of[idx] = b
        slot_of[idx] = count
        heapq.heappush(heap, (load + w_arr[idx], count + 1, b))
    return bin_of, slot_of


def _host_prep(x, edge_index, batch):
    """All O(E) index work in numpy; returns per-core device arrays."""
    import ml_dtypes

    bf16 = ml_dtypes.bfloat16
    fp8 = ml_dtypes.float8_e4m3
    x = np.asarray(x, dtype=np.float32)
    ei = np.asarray(edge_index).astype(np.int64)
    batch = np.asarray(batch).astype(np.int64)
    n = N_NODES

    loops = np.arange(n, dtype=np.int64)
    row = np.concatenate([ei[0], loops])
    col = np.concatenate([ei[1], loops])

    deg = np.bincount(col, minlength=n).astype(np.float64)
    dis = np.where(deg > 0, 1.0 / np.sqrt(np.maximum(deg, 1.0)), 0.0)
    norm = dis[row] * dis[col]                     # fp64

    # layer-1 aggregation (FIN=2) on host
    agg1 = np.empty((n, FIN), dtype=np.float64)
    for f in range(FIN):
        agg1[:, f] = np.bincount(
            col, weights=norm * x[row, f].astype(np.float64), minlength=n
        )

    # ---- node -> (core, window, slot) via LPT packing on indegree+1 ----
    wgt = np.bincount(col, minlength=n)            # includes the self loop
    bin_raw, slot_of = _lpt_pack(wgt)
    # deal bins to (core, window) so similar loads share a window
    loads = np.zeros(NBINS, dtype=np.int64)
    np.add.at(loads, bin_raw, wgt)
    deal = np.argsort(-loads, kind="stable")       # deal[k] = raw bin id
    bin_rank = np.empty(NBINS, dtype=np.int64)
    bin_rank[deal] = np.arange(NBINS)
    rank = bin_rank[bin_raw]                       # 0..783, sorted by load
    node_w = rank // N_CORES                       # window 0..97
    node_c = rank % N_CORES                        # core 0..7

    # ---- edges ordered by (dst core, dst window) ----
    e_rank = rank[col]
    order = np.argsort(e_rank, kind="stable")
    row_s, col_s = row[order], col[order]
    norm_s = norm[order]
    rank_s = e_rank[order]
    c_s = rank_s % N_CORES
    w_s = rank_s // N_CORES

    cnts = np.bincount(e_rank, minlength=NBINS)    # indexed by rank = w*8 + c
    cw_load = cnts.reshape(NW, N_CORES).T          # [core, window]
    T_w = ((cw_load.max(axis=0) + P - 1) // P).astype(np.int64)   # per window
    base_tile = np.concatenate([[0], np.cumsum(T_w)])
    TT = int(base_tile[-1])

    starts = np.concatenate([[0], np.cumsum(cnts)])
    idx_in_bin = np.arange(len(col_s)) - starts[rank_s]
    tile_g = base_tile[w_s] + idx_in_bin // P
    slot = tile_g * P + idx_in_bin % P

    # flat per-slot arrays (norm folded into aT; S is exact one-hot)
    aT = np.zeros((N_CORES, 4, TT * P), dtype=np.float32)
    Sfull = np.zeros((N_CORES, TT * P, P), dtype=bf16)
    aT[c_s, 0, slot] = (agg1[row_s, 0] * norm_s).astype(np.float32)
    aT[c_s, 1, slot] = (agg1[row_s, 1] * norm_s).astype(np.float32)
    aT[c_s, 2, slot] = norm_s.astype(np.float32)
    Sfull[c_s, slot, slot_of[col_s]] = bf16(1.0)
    aTt = aT.reshape(N_CORES, 4, TT, P)
    Sfull = Sfull.reshape(N_CORES, TT, P, P)

    # ---- window-local tile pairing into "blocks" (2 tiles per block) ----
    # block kinds: 1 = full pair, 0 = tail (second slot S-masked to zero)
    nblk_w = ((T_w + 1) // 2).astype(np.int64)     # blocks per window
    base_blk = np.concatenate([[0], np.cumsum(nblk_w)])
    NBLK = int(base_blk[-1])

    # aT with channels replicated 32x along partitions (K=128 L1 matmuls keep
    # the PE array fully active; W1b is scaled by 1/32 to compensate)
    aT4 = np.zeros((N_CORES, 4, NBLK, 2, P), dtype=np.float32)
    S2 = np.zeros((N_CORES, NBLK, P, 2, P), dtype=bf16)
    blk_kind = np.zeros(NBLK, dtype=np.int64)
    for w in range(NW):
        nt = int(T_w[w])
        for b in range(int(nblk_w[w])):
            blk = int(base_blk[w]) + b
            t0 = int(base_tile[w]) + 2 * b
            aT4[:, :, blk, 0, :] = aTt[:, :, t0]
            S2[:, blk, :, 0, :] = Sfull[:, t0]
            if 2 * b + 1 < nt:
                blk_kind[blk] = 1
                aT4[:, :, blk, 1, :] = aTt[:, :, t0 + 1]
                S2[:, blk, :, 1, :] = Sfull[:, t0 + 1]
    aT32 = np.tile(aT4.reshape(N_CORES, 1, 4, NBLK * 2 * P), (1, 32, 1, 1))
    aT32 = aT32.reshape(N_CORES, 128, NBLK * 2 * P)

    # ---- L3: T matrix rows permuted to node home slots ----
    gcol = batch[col]                              # graph of each edge's dst
    Tmat = np.bincount(
        row * G + gcol, weights=norm, minlength=n * G
    ).astype(np.float32).reshape(n, G)
    Tpad = np.zeros((N_CORES, NW * P, G), dtype=bf16)
    Tpad[node_c, node_w * P + slot_of] = Tmat.astype(bf16)

    cnt = np.bincount(batch, minlength=G).astype(np.float32)
    return (aT32.astype(bf16), S2, Tpad, cnt, nblk_w, blk_kind, NBLK, base_blk)


def _build_device_program(NBLK, nblk_w, blk_kind, base_blk, nw=NW):
    import concourse.mybir as mybir
    import concourse.tile as tile
    from concourse import bacc

    f32 = mybir.dt.float32
    bf16 = mybir.dt.bfloat16
    fp8 = mybir.dt.float8e4
    DR = mybir.MatmulPerfMode.DoubleRow
    nc = bacc.Bacc(None, target_bir_lowering=False, debug=False)

    aT_d = nc.dram_tensor("aT", [P, NBLK * 2 * P], bf16, kind="ExternalInput")
    S_d = nc.dram_tensor("S", [NBLK, P, 2, P], bf16, kind="ExternalInput")
    T_d = nc.dram_tensor("T", [NW, P, G], bf16, kind="ExternalInput")
    W1b_d = nc.dram_tensor("W1b", [P, H], bf16, kind="ExternalInput")
    W2_d = nc.dram_tensor("W2", [8, P, H], fp8, kind="ExternalInput")
    b2_d = nc.dram_tensor("b2", [1, H], bf16, kind="ExternalInput")
    out_d = nc.dram_tensor("pg3", [G, H], f32, kind="ExternalOutput")

    CH = 16                      # aT blocks per staged chunk
    n_chunks = (NBLK + CH - 1) // CH
    GS = 8.0                     # g2T fp8 scale

    with tile.TileContext(nc) as tc:
        with (
            tc.tile_pool(name="const", bufs=1) as cst,
            tc.tile_pool(name="sa", bufs=2) as sa,
            tc.tile_pool(name="sS", bufs=10) as sS,
            tc.tile_pool(name="smsg", bufs=8) as smsg,
            tc.tile_pool(name="sg2T", bufs=2) as sg2T,
            tc.tile_pool(name="sh2", bufs=2) as sh2,
            tc.tile_pool(name="sT", bufs=2) as sT,
            tc.tile_pool(name="zp", bufs=4, space="PSUM") as zp,
            tc.tile_pool(name="gp", bufs=2, space="PSUM") as gp,
            tc.tile_pool(name="hp", bufs=2, space="PSUM") as hp,
        ):
            Relu = mybir.ActivationFunctionType.Relu
            Copy = mybir.ActivationFunctionType.Copy
            Mult = mybir.AluOpType.mult
            Max = mybir.AluOpType.max

            # W1b/32 replicated across all 128 partitions (K=128 L1 matmuls)
            W1bd = cst.tile([P, H], bf16, tag="W1bd")
            nc.sync.dma_start(W1bd[:], W1b_d[:])
            W2s = cst.tile([P, 8, H], fp8, tag="W2s")
            nc.sync.dma_start(W2s[:], W2_d[:].rearrange("c p f -> p c f"))
            b2s = cst.tile([1, H], bf16, tag="b2s")
            nc.sync.dma_start(b2s[:], b2_d[:])
            ones1 = cst.tile([1, P], bf16, tag="ones1")
            nc.vector.memset(ones1[:], 1.0)
            pg3s = cst.tile([G, H], f32, tag="pg3s")
            nc.vector.memset(pg3s[:], 0.0)

            chunks = {}          # chunk idx -> staged aT tile
            msg_of = {}          # block -> msg pair tile [P, 2, H]
            Ss_of = {}           # block -> one-hot S pair tile [P, 2, P]

            def stage_chunk(ci):
                if ci >= n_chunks or ci in chunks:
                    return
                t_ = sa.tile([P, CH * 2 * P], bf16, tag="aTc")
                lo = ci * CH * 2 * P
                hi = min((ci + 1) * CH * 2 * P, NBLK * 2 * P)
                nc.sync.dma_start(t_[:, : hi - lo], aT_d[:, lo:hi])
                chunks[ci] = t_

            state = {"b": 0, "tail": 0}

            def emit_block():
                blk = state["b"]
                if blk >= NBLK:
                    return
                state["b"] = blk + 1
                ci, off = blk // CH, (blk % CH) * 2 * P
                if blk % CH == 0:
                    stage_chunk(ci + 1)
                aTc = chunks[ci]
                full = blk_kind[blk] == 1
                Ss = sS.tile([P, 2, P], bf16, tag="Ss")
                nc.sync.dma_start(Ss[:], S_d[blk])
                Ss_of[blk] = Ss
                mp = smsg.tile([P, 2, H], bf16, tag="msg")
                for t_in in range(2 if full else 1):
                    sl = slice(off + t_in * P, off + (t_in + 1) * P)
                    zA = zp.tile([P, 512], f32, tag="z")
                    zB = zp.tile([P, 512], f32, tag="z")
                    nc.tensor.matmul(zA[:], aTc[:, sl], W1bd[:, :512],
                                     start=True, stop=True)
                    nc.tensor.matmul(zB[:], aTc[:, sl], W1bd[:, 512:],
                                     start=True, stop=True)
                    nc.scalar.activation(mp[:, t_in, :512], zA[:], Relu)
                    nc.vector.tensor_scalar_max(mp[:, t_in, 512:], zB[:], 0.0)
                msg_of[blk] = mp

            def emit_block_if(target):
                if state["b"] < min(target, NBLK):
                    emit_block()

            # prologue: window 0's blocks
            stage_chunk(0)
            while state["b"] < int(base_blk[1]):
                emit_block()

            for w in range(nw):
                Tt = sT.tile([P, G], bf16, tag="Tt")
                nc.sync.dma_start(Tt[:], T_d[w])
                nb = int(nblk_w[w])
                b0 = int(base_blk[w])
                target = int(base_blk[min(w + 2, nw)])

                # dual-form bf16 aggregation: g2T[f,dst] += msg[e,f].T @ S[e,dst]
                # (per tile slot; a tail block's unused slot 1 is skipped)
                slots = []
                for b in range(nb):
                    ns = 2 if blk_kind[b0 + b] == 1 else 1
                    slots += [(b0 + b, s_) for s_ in range(ns)]
                g2T = sg2T.tile([P, 8, P], fp8, tag="g2T")
                for p4 in range(4):
                    gA = gp.tile([P, 512], f32, tag="g")
                    gB = gp.tile([P, 512], f32, tag="g")
                    jA, jB = 2 * p4, 2 * p4 + 1
                    for si, (blk, s_) in enumerate(slots):
                        st_, sp_ = si == 0, si == len(slots) - 1
                        nc.tensor.matmul(
                            gA[:, :P], msg_of[blk][:, s_, jA * P : (jA + 1) * P],
                            Ss_of[blk][:, s_, :], start=st_, stop=sp_,
                        )
                        nc.tensor.matmul(
                            gB[:, :P], msg_of[blk][:, s_, jB * P : (jB + 1) * P],
                            Ss_of[blk][:, s_, :], start=st_, stop=sp_,
                        )
                    nc.scalar.activation(g2T[:, jA], gA[:, :P], Copy, scale=GS)
                    nc.vector.tensor_scalar_mul(g2T[:, jB], gB[:, :P], GS)
                    if p4 < 3:
                        emit_block_if(target)   # cover gp drain w/ L1 stream

                # h2 = relu((g2*GS @ W2*16)/128 + b2); fp8 DoubleRow pairs
                hps = []
                for half in range(2):
                    lo = half * 512
                    h2p = hp.tile([P, 512], f32, tag="h")
                    for j2 in range(4):
                        nc.tensor.matmul(
                            h2p[:], g2T[:, 2 * j2 : 2 * j2 + 2, :],
                            W2s[:, 2 * j2 : 2 * j2 + 2, lo : lo + 512],
                            start=(j2 == 0), stop=False, perf_mode=DR,
                        )
                    nc.tensor.matmul(
                        h2p[:], ones1[:1, :], b2s[:1, lo : lo + 512],
                        start=False, stop=True,
                    )
                    hps.append(h2p)
                    if half == 0:
                        emit_block_if(target)
                h2b = sh2.tile([P, H], bf16, tag="h2b")
                nc.scalar.activation(h2b[:, :512], hps[0][:], Relu,
                                     scale=1.0 / 128)
                nc.vector.tensor_scalar(
                    h2b[:, 512:], hps[1][:], 1.0 / 128, 0.0,
                    op0=Mult, op1=Max,
                )
                emit_block_if(target)
                for half in range(2):
                    lo = half * 512
                    cp = hp.tile([P, 512], f32, tag="h")
                    nc.tensor.matmul(
                        cp[:], Tt[:], h2b[:, lo : lo + 512], start=True, stop=True
                    )
                    nc.vector.tensor_add(
                        pg3s[:, lo : lo + 512], pg3s[:, lo : lo + 512], cp[:]
                    )
                for b in range(nb):
                    msg_of.pop(b0 + b, None)
                    Ss_of.pop(b0 + b, None)

            nc.sync.dma_start(out_d[:], pg3s[:])

    nc.finalize()
    return nc


def kernel(x, W1, b1, W2, b2, W3, b3, Wlin, blin, edge_index, batch, num_graphs):
    import ml_dtypes
    from concourse.bass_utils import run_bass_kernel_spmd

    bf16 = ml_dtypes.bfloat16
    fp8 = ml_dtypes.float8_e4m3
    x = np.asarray(x, dtype=np.float32)
    W1 = np.asarray(W1, dtype=np.float32)
    b1 = np.asarray(b1, dtype=np.float32)
    W2 = np.asarray(W2, dtype=np.float32)
    b2 = np.asarray(b2, dtype=np.float32)
    W3 = np.asarray(W3, dtype=np.float32)
    b3 = np.asarray(b3, dtype=np.float32)
    Wlin = np.asarray(Wlin, dtype=np.float32)
    blin = np.asarray(blin, dtype=np.float32)

    (aT4, S2, Tpad, cnt, nblk_w, blk_kind, NBLK, base_blk) = _host_prep(
        x, edge_index, batch
    )

    nc = _build_device_program(NBLK, nblk_w, blk_kind, base_blk)

    W1b = np.zeros((4, H), dtype=np.float32)
    W1b[:2] = W1
    W1b[2] = b1
    W1b = np.tile(W1b / 32.0, (32, 1)).astype(bf16)   # [128, H]
    W2r = np.ascontiguousarray((W2 * 16.0).reshape(8, P, H)).astype(fp8)
    b2r = (b2 * 128.0).reshape(1, H).astype(bf16)

    in_maps = [
        {
            "aT": np.ascontiguousarray(aT4[c]),
            "S": np.ascontiguousarray(S2[c]),
            "T": np.ascontiguousarray(Tpad[c].reshape(NW, P, G)),
            "W1b": W1b,
            "W2": W2r,
            "b2": b2r,
        }
        for c in range(N_CORES)
    ]
    res = run_bass_kernel_spmd(nc, in_maps, core_ids=list(range(N_CORES)))
    global LAST_RESULTS
    LAST_RESULTS = res
    pg3 = np.zeros((G, H), dtype=np.float64)
    for r in res.results:
        pg3 += r["pg3"].astype(np.float64)
    pg3 = pg3.astype(np.float32)

    pooled = (pg3 @ W3 + cnt[:, None] * b3[None, :]) / np.maximum(cnt, 1.0)[:, None]
    out = pooled @ Wlin + blin[None, :]
    return out.astype(np.float32)


# revision 44
# speedup vs baseline: 1.7929x; 1.0555x over previous
"""GCN (3-layer, PyG-style) forward on 8 Trainium2 NeuronCores.

Math restructuring
------------------
reference:
  h1 = relu(Anorm @ x @ W1 + b1)          (Anorm includes self loops + sym norm)
  h2 = relu(Anorm @ h1 @ W2 + b2)
  h3 = Anorm @ h2 @ W3 + b3
  out = segment_mean(h3, batch) @ Wlin + blin

Because GCNConv aggregation and the weight matmul commute, and pooling is
linear, this is equivalent to:
  agg1 = Anorm @ x                        # [N,2]  (tiny -> host)
  msg_e = relu(norm_e * (agg1[src_e] @ W1 + b1))    # per-edge (norm>0 commutes
                                                    #  through relu)
  g2   = scatter-sum msg to dst           # exact one-hot matmul on device
  h2   = relu(g2 @ W2 + b2)               # dense matmul on device
  pg3[g] = sum_n T[n,g] * h2[n]           # T[n,g] = sum of norm over n's
                                          #  out-edges into graph g
  out  = ((pg3 @ W3 + cnt*b3)/max(cnt,1)) @ Wlin + blin   # [128,1024] -> host

Sharding: nodes are LPT bin-packed into 8 cores x 98 windows of 128 slots so
that each (core, window) bin holds ~638 incident edges (load-balanced).  Every
core runs the same program (SPMD) on its own edge arrays, padded to identical
tile counts.  Per-core output is a partial pg3 [128,1024]; the host sums them
(the "all-reduce").

Device-side structure:
 - L1 (K=4, bf16) matmuls are 4-way row-group packed via tile_position: aT and
   W1b are duplicated at SBUF base partitions {0,32,64,96} so the four matmuls
   of an edge-tile pair run concurrently in separate 32-row strips
 - messages are stored fp8e4m3 (x16); the aggregation runs in "dual form"
   (g2T[f,dst] += msg[e,f].T @ S[e,dst]) as fp8 DoubleRow matmuls that pack
   two edge tiles per instruction; S is an exact one-hot so a window's odd
   tail tile is paired with an all-zero S column (masking a dummy slot)
 - h2 = relu(g2 @ W2 + b2) also runs as fp8 DoubleRow (g2T x8, W2 x16,
   rescaled 1/128 inside the relu); pg3 accumulation is bf16->fp32
 - L1 work for window w+1 is interleaved between the aggregation passes of
   window w, keeping every matmul's dependencies one window ahead (dense PE
   stream, HAM stays warm) and covering the PSUM drain latencies
"""

import numpy as np

LAST_RESULTS = None  # set by kernel() for test harness introspection

N_NODES = 100000
N_EDGES = 400000
G = 128
FIN = 2
H = 1024
N_CORES = 8
P = 128
NW = 98                      # windows per core (98*128 = 12544 >= 12500 slots)
NBINS = N_CORES * NW


def _lpt_pack(wgt):
    """Assign each node to one of 784 (core,window) bins, balancing total
    edge weight per bin with a <=128 nodes/bin cap.  Returns (bin_of, slot_of).
    """
    import heapq

    n = len(wgt)
    order = np.argsort(-wgt, kind="stable")
    heap = [(0, 0, b) for b in range(NBINS)]
    heapq.heapify(heap)
    bin_of = np.empty(n, dtype=np.int64)
    slot_of = np.empty(n, dtype=np.int64)
    w_arr = wgt.tolist()
    for idx in order.tolist():
        while True:
            load, count, b = heapq.heappop(heap)
            if count < P:
                break
        bin_of[idx] = b
        slot_of[idx] = count
        heapq.heappush(heap, (load + w_arr[idx], count + 1, b))
    return bin_of, slot_of


def _host_prep(x, edge_index, batch):
    """All O(E) index work in numpy; returns per-core device arrays."""
    import ml_dtypes

    bf16 = ml_dtypes.bfloat16
    fp8 = ml_dtypes.float8_e4m3
    x = np.asarray(x, dtype=np.float32)
    ei = np.asarray(edge_index).astype(np.int64)
    batch = np.asarray(batch).astype(np.int64)
    n = N_NODES

    loops = np.arange(n, dtype=np.int64)
    row = np.concatenate([ei[0], loops])
    col = np.concatenate([ei[1], loops])

    deg = np.bincount(col, minlength=n).astype(np.float64)
    dis = np.where(deg > 0, 1.0 / np.sqrt(np.maximum(deg, 1.0)), 0.0)
    norm = dis[row] * dis[col]                     # fp64

    # layer-1 aggregation (FIN=2) on host
    agg1 = np.empty((n, FIN), dtype=np.float64)
    for f in range(FIN):
        agg1[:, f] = np.bincount(
            col, weights=norm * x[row, f].astype(np.float64), minlength=n
        )

    # ---- node -> (core, window, slot) via LPT packing on indegree+1 ----
    wgt = np.bincount(col, minlength=n)            # includes the self loop
    bin_raw, slot_of = _lpt_pack(wgt)
    # deal bins to (core, window) so similar loads share a window
    loads = np.zeros(NBINS, dtype=np.int64)
    np.add.at(loads, bin_raw, wgt)
    deal = np.argsort(-loads, kind="stable")       # deal[k] = raw bin id
    bin_rank = np.empty(NBINS, dtype=np.int64)
    bin_rank[deal] = np.arange(NBINS)
    rank = bin_rank[bin_raw]                       # 0..783, sorted by load
    node_w = rank // N_CORES                       # window 0..97
    node_c = rank % N_CORES                        # core 0..7

    # ---- edges ordered by (dst core, dst window) ----
    e_rank = rank[col]
    order = np.argsort(e_rank, kind="stable")
    row_s, col_s = row[order], col[order]
    norm_s = norm[order]
    rank_s = e_rank[order]
    c_s = rank_s % N_CORES
    w_s = rank_s // N_CORES

    cnts = np.bincount(e_rank, minlength=NBINS)    # indexed by rank = w*8 + c
    cw_load = cnts.reshape(NW, N_CORES).T          # [core, window]
    T_w = ((cw_load.max(axis=0) + P - 1) // P).astype(np.int64)   # per window
    base_tile = np.concatenate([[0], np.cumsum(T_w)])
    TT = int(base_tile[-1])

    starts = np.concatenate([[0], np.cumsum(cnts)])
    idx_in_bin = np.arange(len(col_s)) - starts[rank_s]
    tile_g = base_tile[w_s] + idx_in_bin // P
    slot = tile_g * P + idx_in_bin % P

    # flat per-slot arrays (norm folded into aT; S is exact one-hot)
    aT = np.zeros((N_CORES, 4, TT * P), dtype=np.float32)
    Sfull = np.zeros((N_CORES, TT * P, P), dtype=bf16)
    aT[c_s, 0, slot] = (agg1[row_s, 0] * norm_s).astype(np.float32)
    aT[c_s, 1, slot] = (agg1[row_s, 1] * norm_s).astype(np.float32)
    aT[c_s, 2, slot] = norm_s.astype(np.float32)
    Sfull[c_s, slot, slot_of[col_s]] = bf16(1.0)
    aTt = aT.reshape(N_CORES, 4, TT, P)
    Sfull = Sfull.reshape(N_CORES, TT, P, P)

    # ---- window-local tile pairing into "blocks" (2 tiles per block) ----
    # block kinds: 1 = full pair, 0 = tail (second slot S-masked to zero)
    nblk_w = ((T_w + 1) // 2).astype(np.int64)     # blocks per window
    base_blk = np.concatenate([[0], np.cumsum(nblk_w)])
    NBLK = int(base_blk[-1])

    # aT with channels replicated 32x along partitions (K=128 L1 matmuls keep
    # the PE array fully active; W1b is scaled by 1/32 to compensate)
    aT4 = np.zeros((N_CORES, 4, NBLK, 2, P), dtype=np.float32)
    S2 = np.zeros((N_CORES, NBLK, P, 2, P), dtype=bf16)
    blk_kind = np.zeros(NBLK, dtype=np.int64)
    for w in range(NW):
        nt = int(T_w[w])
        for b in range(int(nblk_w[w])):
            blk = int(base_blk[w]) + b
            t0 = int(base_tile[w]) + 2 * b
            aT4[:, :, blk, 0, :] = aTt[:, :, t0]
            S2[:, blk, :, 0, :] = Sfull[:, t0]
            if 2 * b + 1 < nt:
                blk_kind[blk] = 1
                aT4[:, :, blk, 1, :] = aTt[:, :, t0 + 1]
                S2[:, blk, :, 1, :] = Sfull[:, t0 + 1]
    aT32 = np.tile(aT4.reshape(N_CORES, 1, 4, NBLK * 2 * P), (1, 32, 1, 1))
    aT32 = aT32.reshape(N_CORES, 128, NBLK * 2 * P)

    # ---- L3: T matrix rows permuted to node home slots ----
    gcol = batch[col]                              # graph of each edge's dst
    Tmat = np.bincount(
        row * G + gcol, weights=norm, minlength=n * G
    ).astype(np.float32).reshape(n, G)
    Tpad = np.zeros((N_CORES, NW * P, G), dtype=bf16)
    Tpad[node_c, node_w * P + slot_of] = Tmat.astype(bf16)

    cnt = np.bincount(batch, minlength=G).astype(np.float32)
    return (aT32.astype(bf16), S2, Tpad, cnt, nblk_w, blk_kind, NBLK, base_blk)


def _build_device_program(NBLK, nblk_w, blk_kind, base_blk, nw=NW):
    import concourse.mybir as mybir
    import concourse.tile as tile
    from concourse import bacc

    f32 = mybir.dt.float32
    bf16 = mybir.dt.bfloat16
    fp8 = mybir.dt.float8e4
    DR = mybir.MatmulPerfMode.DoubleRow
    nc = bacc.Bacc(None, target_bir_lowering=False, debug=False)

    aT_d = nc.dram_tensor("aT", [P, NBLK * 2 * P], bf16, kind="ExternalInput")
    S_d = nc.dram_tensor("S", [NBLK, P, 2, P], bf16, kind="ExternalInput")
    T_d = nc.dram_tensor("T", [NW, P, G], bf16, kind="ExternalInput")
    W1b_d = nc.dram_tensor("W1b", [P, H], bf16, kind="ExternalInput")
    W2_d = nc.dram_tensor("W2", [8, P, H], fp8, kind="ExternalInput")
    b2_d = nc.dram_tensor("b2", [1, H], bf16, kind="ExternalInput")
    out_d = nc.dram_tensor("pg3", [G, H], f32, kind="ExternalOutput")

    CH = 16                      # aT blocks per staged chunk
    n_chunks = (NBLK + CH - 1) // CH
    GS = 8.0                     # g2T fp8 scale

    with tile.TileContext(nc) as tc:
        with (
            tc.tile_pool(name="const", bufs=1) as cst,
            tc.tile_pool(name="sa", bufs=2) as sa,
            tc.tile_pool(name="sS", bufs=10) as sS,
            tc.tile_pool(name="smsg", bufs=8) as smsg,
            tc.tile_pool(name="sg2T", bufs=2) as sg2T,
            tc.tile_pool(name="sh2", bufs=2) as sh2,
            tc.tile_pool(name="sT", bufs=2) as sT,
            tc.tile_pool(name="zp", bufs=4, space="PSUM") as zp,
            tc.tile_pool(name="gp", bufs=2, space="PSUM") as gp,
            tc.tile_pool(name="hp", bufs=2, space="PSUM") as hp,
        ):
            Relu = mybir.ActivationFunctionType.Relu
            Copy = mybir.ActivationFunctionType.Copy
            Mult = mybir.AluOpType.mult
            Max = mybir.AluOpType.max

            # W1b/32 replicated across all 128 partitions (K=128 L1 matmuls)
            W1bd = cst.tile([P, H], bf16, tag="W1bd")
            nc.sync.dma_start(W1bd[:], W1b_d[:])
            W2s = cst.tile([P, 8, H], fp8, tag="W2s")
            b2s = cst.tile([1, H], bf16, tag="b2s")
            ones1 = cst.tile([1, P], bf16, tag="ones1")
            nc.vector.memset(ones1[:], 1.0)
            pg3s = cst.tile([G, H], f32, tag="pg3s")
            nc.vector.memset(pg3s[:], 0.0)

            chunks = {}          # chunk idx -> staged aT tile
            msg_of = {}          # block -> msg pair tile [P, 2, H]
            Ss_of = {}           # block -> one-hot S pair tile [P, 2, P]

            def stage_chunk(ci):
                if ci >= n_chunks or ci in chunks:
                    return
                t_ = sa.tile([P, CH * 2 * P], bf16, tag="aTc")
                lo = ci * CH * 2 * P
                hi = min((ci + 1) * CH * 2 * P, NBLK * 2 * P)
                nc.sync.dma_start(t_[:, : hi - lo], aT_d[:, lo:hi])
                chunks[ci] = t_

            state = {"b": 0, "tail": 0}

            def emit_block():
                blk = state["b"]
                if blk >= NBLK:
                    return
                state["b"] = blk + 1
                ci, off = blk // CH, (blk % CH) * 2 * P
                if blk % CH == 0:
                    stage_chunk(ci + 1)
                aTc = chunks[ci]
                full = blk_kind[blk] == 1
                Ss = sS.tile([P, 2, P], bf16, tag="Ss")
                nc.sync.dma_start(Ss[:], S_d[blk])
                Ss_of[blk] = Ss
                mp = smsg.tile([P, 2, H], bf16, tag="msg")
                for t_in in range(2 if full else 1):
                    sl = slice(off + t_in * P, off + (t_in + 1) * P)
                    zA = zp.tile([P, 512], f32, tag="z")
                    zB = zp.tile([P, 512], f32, tag="z")
                    nc.tensor.matmul(zA[:], aTc[:, sl], W1bd[:, :512],
                                     start=True, stop=True)
                    nc.tensor.matmul(zB[:], aTc[:, sl], W1bd[:, 512:],
                                     start=True, stop=True)
                    nc.scalar.activation(mp[:, t_in, :512], zA[:], Relu)
                    nc.vector.tensor_scalar_max(mp[:, t_in, 512:], zB[:], 0.0)
                msg_of[blk] = mp

            def emit_block_if(target):
                if state["b"] < min(target, NBLK):
                    emit_block()

            # prologue: window 0's blocks first, then the bulk constants
            # (W2 isn't needed until the first h2, ~10us in)
            stage_chunk(0)
            while state["b"] < int(base_blk[1]):
                emit_block()
            nc.sync.dma_start(W2s[:], W2_d[:].rearrange("c p f -> p c f"))
            nc.sync.dma_start(b2s[:], b2_d[:])

            for w in range(nw):
                Tt = sT.tile([P, G], bf16, tag="Tt")
                nc.sync.dma_start(Tt[:], T_d[w])
                nb = int(nblk_w[w])
                b0 = int(base_blk[w])
                target = int(base_blk[min(w + 2, nw)])

                # dual-form bf16 aggregation: g2T[f,dst] += msg[e,f].T @ S[e,dst]
                # (per tile slot; a tail block's unused slot 1 is skipped)
                slots = []
                for b in range(nb):
                    ns = 2 if blk_kind[b0 + b] == 1 else 1
                    slots += [(b0 + b, s_) for s_ in range(ns)]
                g2T = sg2T.tile([P, 8, P], fp8, tag="g2T")
                for p4 in range(4):
                    gA = gp.tile([P, 512], f32, tag="g")
                    gB = gp.tile([P, 512], f32, tag="g")
                    jA, jB = 2 * p4, 2 * p4 + 1
                    for si, (blk, s_) in enumerate(slots):
                        st_, sp_ = si == 0, si == len(slots) - 1
                        nc.tensor.matmul(
                            gA[:, :P], msg_of[blk][:, s_, jA * P : (jA + 1) * P],
                            Ss_of[blk][:, s_, :], start=st_, stop=sp_,
                        )
                        nc.tensor.matmul(
                            gB[:, :P], msg_of[blk][:, s_, jB * P : (jB + 1) * P],
                            Ss_of[blk][:, s_, :], start=st_, stop=sp_,
                        )
                    nc.scalar.activation(g2T[:, jA], gA[:, :P], Copy, scale=GS)
                    nc.vector.tensor_scalar_mul(g2T[:, jB], gB[:, :P], GS)
                    if p4 < 3:
                        emit_block_if(target)   # cover gp drain w/ L1 stream

                # h2 = relu((g2*GS @ W2*16)/128 + b2); fp8 DoubleRow pairs
                hps = []
                for half in range(2):
                    lo = half * 512
                    h2p = hp.tile([P, 512], f32, tag="h")
                    for j2 in range(4):
                        nc.tensor.matmul(
                            h2p[:], g2T[:, 2 * j2 : 2 * j2 + 2, :],
                            W2s[:, 2 * j2 : 2 * j2 + 2, lo : lo + 512],
                            start=(j2 == 0), stop=False, perf_mode=DR,
                        )
                    nc.tensor.matmul(
                        h2p[:], ones1[:1, :], b2s[:1, lo : lo + 512],
                        start=False, stop=True,
                    )
                    hps.append(h2p)
                    if half == 0:
                        emit_block_if(target)
                h2b = sh2.tile([P, H], bf16, tag="h2b")
                nc.scalar.activation(h2b[:, :512], hps[0][:], Relu,
                                     scale=1.0 / 128)
                nc.scalar.activation(h2b[:, 512:], hps[1][:], Relu,
                                     scale=1.0 / 128)
                emit_block_if(target)
                for half in range(2):
                    lo = half * 512
                    cp = hp.tile([P, 512], f32, tag="h")
                    nc.tensor.matmul(
                        cp[:], Tt[:], h2b[:, lo : lo + 512], start=True, stop=True
                    )
                    nc.vector.tensor_add(
                        pg3s[:, lo : lo + 512], pg3s[:, lo : lo + 512], cp[:]
                    )
                for b in range(nb):
                    msg_of.pop(b0 + b, None)
                    Ss_of.pop(b0 + b, None)

            nc.sync.dma_start(out_d[:], pg3s[:])

    nc.finalize()
    return nc


def kernel(x, W1, b1, W2, b2, W3, b3, Wlin, blin, edge_index, batch, num_graphs):
    import ml_dtypes
    from concourse.bass_utils import run_bass_kernel_spmd

    bf16 = ml_dtypes.bfloat16
    fp8 = ml_dtypes.float8_e4m3
    x = np.asarray(x, dtype=np.float32)
    W1 = np.asarray(W1, dtype=np.float32)
    b1 = np.asarray(b1, dtype=np.float32)
    W2 = np.asarray(W2, dtype=np.float32)
    b2 = np.asarray(b2, dtype=np.float32)
    W3 = np.asarray(W3, dtype=np.float32)
    b3 = np.asarray(b3, dtype=np.float32)
    Wlin = np.asarray(Wlin, dtype=np.float32)
    blin = np.asarray(blin, dtype=np.float32)

    (aT4, S2, Tpad, cnt, nblk_w, blk_kind, NBLK, base_blk) = _host_prep(
        x, edge_index, batch
    )

    nc = _build_device_program(NBLK, nblk_w, blk_kind, base_blk)

    W1b = np.zeros((4, H), dtype=np.float32)
    W1b[:2] = W1
    W1b[2] = b1
    W1b = np.tile(W1b / 32.0, (32, 1)).astype(bf16)   # [128, H]
    W2r = np.ascontiguousarray((W2 * 16.0).reshape(8, P, H)).astype(fp8)
    b2r = (b2 * 128.0).reshape(1, H).astype(bf16)

    in_maps = [
        {
            "aT": np.ascontiguousarray(aT4[c]),
            "S": np.ascontiguousarray(S2[c]),
            "T": np.ascontiguousarray(Tpad[c].reshape(NW, P, G)),
            "W1b": W1b,
            "W2": W2r,
            "b2": b2r,
        }
        for c in range(N_CORES)
    ]
    res = run_bass_kernel_spmd(nc, in_maps, core_ids=list(range(N_CORES)))
    global LAST_RESULTS
    LAST_RESULTS = res
    pg3 = np.zeros((G, H), dtype=np.float64)
    for r in res.results:
        pg3 += r["pg3"].astype(np.float64)
    pg3 = pg3.astype(np.float32)

    pooled = (pg3 @ W3 + cnt[:, None] * b3[None, :]) / np.maximum(cnt, 1.0)[:, None]
    out = pooled @ Wlin + blin[None, :]
    return out.astype(np.float32)


# revision 50
# speedup vs baseline: 1.8288x; 1.0200x over previous
"""GCN (3-layer, PyG-style) forward on 8 Trainium2 NeuronCores.

Math restructuring
------------------
reference:
  h1 = relu(Anorm @ x @ W1 + b1)          (Anorm includes self loops + sym norm)
  h2 = relu(Anorm @ h1 @ W2 + b2)
  h3 = Anorm @ h2 @ W3 + b3
  out = segment_mean(h3, batch) @ Wlin + blin

Because GCNConv aggregation and the weight matmul commute, and pooling is
linear, this is equivalent to:
  agg1 = Anorm @ x                        # [N,2]  (tiny -> host)
  msg_e = relu(norm_e * (agg1[src_e] @ W1 + b1))    # per-edge (norm>0 commutes
                                                    #  through relu)
  g2   = scatter-sum msg to dst           # exact one-hot matmul on device
  h2   = relu(g2 @ W2 + b2)               # dense matmul on device
  pg3[g] = sum_n T[n,g] * h2[n]           # T[n,g] = sum of norm over n's
                                          #  out-edges into graph g
  out  = ((pg3 @ W3 + cnt*b3)/max(cnt,1)) @ Wlin + blin   # [128,1024] -> host

Sharding: nodes are LPT bin-packed into 8 cores x 98 windows of 128 slots so
that each (core, window) bin holds ~638 incident edges (load-balanced).  Every
core runs the same program (SPMD) on its own edge arrays, padded to identical
tile counts.  Per-core output is a partial pg3 [128,1024]; the host sums them
(the "all-reduce").

Device-side structure:
 - L1 (K=4, bf16) matmuls are 4-way row-group packed via tile_position: aT and
   W1b are duplicated at SBUF base partitions {0,32,64,96} so the four matmuls
   of an edge-tile pair run concurrently in separate 32-row strips
 - messages are stored fp8e4m3 (x16); the aggregation runs in "dual form"
   (g2T[f,dst] += msg[e,f].T @ S[e,dst]) as fp8 DoubleRow matmuls that pack
   two edge tiles per instruction; S is an exact one-hot so a window's odd
   tail tile is paired with an all-zero S column (masking a dummy slot)
 - h2 = relu(g2 @ W2 + b2) also runs as fp8 DoubleRow (g2T x8, W2 x16,
   rescaled 1/128 inside the relu); pg3 accumulation is bf16->fp32
 - L1 work for window w+1 is interleaved between the aggregation passes of
   window w, keeping every matmul's dependencies one window ahead (dense PE
   stream, HAM stays warm) and covering the PSUM drain latencies
"""

import numpy as np

LAST_RESULTS = None  # set by kernel() for test harness introspection

N_NODES = 100000
N_EDGES = 400000
G = 128
FIN = 2
H = 1024
N_CORES = 8
P = 128
NW = 98                      # windows per core (98*128 = 12544 >= 12500 slots)
NBINS = N_CORES * NW


def _lpt_pack(wgt):
    """Assign each node to one of 784 (core,window) bins, balancing total
    edge weight per bin with a <=128 nodes/bin cap.  Returns (bin_of, slot_of).
    """
    import heapq

    n = len(wgt)
    order = np.argsort(-wgt, kind="stable")
    heap = [(0, 0, b) for b in range(NBINS)]
    heapq.heapify(heap)
    bin_of = np.empty(n, dtype=np.int64)
    slot_of = np.empty(n, dtype=np.int64)
    w_arr = wgt.tolist()
    for idx in order.tolist():
        while True:
            load, count, b = heapq.heappop(heap)
            if count < P:
                break
        bin_of[idx] = b
        slot_of[idx] = count
        heapq.heappush(heap, (load + w_arr[idx], count + 1, b))
    return bin_of, slot_of


def _host_prep(x, edge_index, batch):
    """All O(E) index work in numpy; returns per-core device arrays."""
    import ml_dtypes

    bf16 = ml_dtypes.bfloat16
    fp8 = ml_dtypes.float8_e4m3
    x = np.asarray(x, dtype=np.float32)
    ei = np.asarray(edge_index).astype(np.int64)
    batch = np.asarray(batch).astype(np.int64)
    n = N_NODES

    loops = np.arange(n, dtype=np.int64)
    row = np.concatenate([ei[0], loops])
    col = np.concatenate([ei[1], loops])

    deg = np.bincount(col, minlength=n).astype(np.float64)
    dis = np.where(deg > 0, 1.0 / np.sqrt(np.maximum(deg, 1.0)), 0.0)
    norm = dis[row] * dis[col]                     # fp64

    # layer-1 aggregation (FIN=2) on host
    agg1 = np.empty((n, FIN), dtype=np.float64)
    for f in range(FIN):
        agg1[:, f] = np.bincount(
            col, weights=norm * x[row, f].astype(np.float64), minlength=n
        )

    # ---- node -> (core, window, slot) via LPT packing on indegree+1 ----
    wgt = np.bincount(col, minlength=n)            # includes the self loop
    bin_raw, slot_of = _lpt_pack(wgt)
    # deal bins to (core, window) so similar loads share a window
    loads = np.zeros(NBINS, dtype=np.int64)
    np.add.at(loads, bin_raw, wgt)
    deal = np.argsort(-loads, kind="stable")       # deal[k] = raw bin id
    bin_rank = np.empty(NBINS, dtype=np.int64)
    bin_rank[deal] = np.arange(NBINS)
    rank = bin_rank[bin_raw]                       # 0..783, sorted by load
    node_w = rank // N_CORES                       # window 0..97
    node_c = rank % N_CORES                        # core 0..7

    # ---- edges ordered by (dst core, dst window) ----
    e_rank = rank[col]
    order = np.argsort(e_rank, kind="stable")
    row_s, col_s = row[order], col[order]
    norm_s = norm[order]
    rank_s = e_rank[order]
    c_s = rank_s % N_CORES
    w_s = rank_s // N_CORES

    cnts = np.bincount(e_rank, minlength=NBINS)    # indexed by rank = w*8 + c
    cw_load = cnts.reshape(NW, N_CORES).T          # [core, window]
    T_w = ((cw_load.max(axis=0) + P - 1) // P).astype(np.int64)   # per window
    base_tile = np.concatenate([[0], np.cumsum(T_w)])
    TT = int(base_tile[-1])

    starts = np.concatenate([[0], np.cumsum(cnts)])
    idx_in_bin = np.arange(len(col_s)) - starts[rank_s]
    tile_g = base_tile[w_s] + idx_in_bin // P
    slot = tile_g * P + idx_in_bin % P

    # flat per-slot arrays (norm folded into aT; S is exact one-hot)
    aT = np.zeros((N_CORES, 4, TT * P), dtype=np.float32)
    Sfull = np.zeros((N_CORES, TT * P, P), dtype=bf16)
    aT[c_s, 0, slot] = (agg1[row_s, 0] * norm_s).astype(np.float32)
    aT[c_s, 1, slot] = (agg1[row_s, 1] * norm_s).astype(np.float32)
    aT[c_s, 2, slot] = norm_s.astype(np.float32)
    Sfull[c_s, slot, slot_of[col_s]] = bf16(1.0)
    aTt = aT.reshape(N_CORES, 4, TT, P)
    Sfull = Sfull.reshape(N_CORES, TT, P, P)

    # ---- window-local tile pairing into "blocks" (2 tiles per block) ----
    # block kinds: 1 = full pair, 0 = tail (second slot S-masked to zero)
    nblk_w = ((T_w + 1) // 2).astype(np.int64)     # blocks per window
    base_blk = np.concatenate([[0], np.cumsum(nblk_w)])
    NBLK = int(base_blk[-1])

    # aT with channels replicated 32x along partitions (K=128 L1 matmuls keep
    # the PE array fully active; W1b is scaled by 1/32 to compensate)
    aT4 = np.zeros((N_CORES, 4, NBLK, 2, P), dtype=np.float32)
    S2 = np.zeros((N_CORES, NBLK, P, 2, P), dtype=bf16)
    blk_kind = np.zeros(NBLK, dtype=np.int64)
    for w in range(NW):
        nt = int(T_w[w])
        for b in range(int(nblk_w[w])):
            blk = int(base_blk[w]) + b
            t0 = int(base_tile[w]) + 2 * b
            aT4[:, :, blk, 0, :] = aTt[:, :, t0]
            S2[:, blk, :, 0, :] = Sfull[:, t0]
            if 2 * b + 1 < nt:
                blk_kind[blk] = 1
                aT4[:, :, blk, 1, :] = aTt[:, :, t0 + 1]
                S2[:, blk, :, 1, :] = Sfull[:, t0 + 1]
    aT32 = np.tile(aT4.reshape(N_CORES, 1, 4, NBLK * 2 * P), (1, 32, 1, 1))
    aT32 = aT32.reshape(N_CORES, 128, NBLK * 2 * P)

    # ---- L3: T matrix rows permuted to node home slots (fp8, x8, paired) ----
    gcol = batch[col]                              # graph of each edge's dst
    Tmat = np.bincount(
        row * G + gcol, weights=norm, minlength=n * G
    ).astype(np.float32).reshape(n, G)
    Tpad = np.zeros((N_CORES, NW * P, G), dtype=fp8)
    Tpad[node_c, node_w * P + slot_of] = (Tmat * 8.0).astype(fp8)
    # pair consecutive windows: [NW//2, P, 2, G]
    Tpad = Tpad.reshape(N_CORES, NW // 2, 2, P, G).transpose(0, 1, 3, 2, 4)
    Tpad = np.ascontiguousarray(Tpad)

    cnt = np.bincount(batch, minlength=G).astype(np.float32)
    return (aT32.astype(bf16), S2, Tpad, cnt, nblk_w, blk_kind, NBLK, base_blk)


def _build_device_program(NBLK, nblk_w, blk_kind, base_blk, nw=NW):
    import concourse.mybir as mybir
    import concourse.tile as tile
    from concourse import bacc

    f32 = mybir.dt.float32
    bf16 = mybir.dt.bfloat16
    fp8 = mybir.dt.float8e4
    DR = mybir.MatmulPerfMode.DoubleRow
    nc = bacc.Bacc(None, target_bir_lowering=False, debug=False)

    aT_d = nc.dram_tensor("aT", [P, NBLK * 2 * P], bf16, kind="ExternalInput")
    S_d = nc.dram_tensor("S", [NBLK, P, 2, P], bf16, kind="ExternalInput")
    T_d = nc.dram_tensor("T", [NW // 2, P, 2, G], fp8, kind="ExternalInput")
    W1b_d = nc.dram_tensor("W1b", [P, H], bf16, kind="ExternalInput")
    W2_d = nc.dram_tensor("W2", [8, P, H], fp8, kind="ExternalInput")
    b2_d = nc.dram_tensor("b2", [1, H], bf16, kind="ExternalInput")
    out_d = nc.dram_tensor("pg3", [G, H], f32, kind="ExternalOutput")

    CH = 16                      # aT blocks per staged chunk
    n_chunks = (NBLK + CH - 1) // CH
    GS = 8.0                     # g2T fp8 scale

    with tile.TileContext(nc) as tc:
        with (
            tc.tile_pool(name="const", bufs=1) as cst,
            tc.tile_pool(name="sa", bufs=2) as sa,
            tc.tile_pool(name="sS", bufs=10) as sS,
            tc.tile_pool(name="smsg", bufs=8) as smsg,
            tc.tile_pool(name="sg2T", bufs=2) as sg2T,
            tc.tile_pool(name="sh2", bufs=2) as sh2,
            tc.tile_pool(name="sT", bufs=2) as sT,
            tc.tile_pool(name="zp", bufs=4, space="PSUM") as zp,
            tc.tile_pool(name="gp", bufs=2, space="PSUM") as gp,
            tc.tile_pool(name="hp", bufs=2, space="PSUM") as hp,
        ):
            Relu = mybir.ActivationFunctionType.Relu
            Copy = mybir.ActivationFunctionType.Copy
            Mult = mybir.AluOpType.mult
            Max = mybir.AluOpType.max

            # W1b/32 replicated across all 128 partitions (K=128 L1 matmuls)
            W1bd = cst.tile([P, H], bf16, tag="W1bd")
            nc.sync.dma_start(W1bd[:], W1b_d[:])
            W2s = cst.tile([P, 8, H], fp8, tag="W2s")
            b2s = cst.tile([1, H], bf16, tag="b2s")
            ones1 = cst.tile([1, P], bf16, tag="ones1")
            nc.vector.memset(ones1[:], 1.0)
            pg3s = cst.tile([G, H], f32, tag="pg3s")
            nc.vector.memset(pg3s[:], 0.0)

            chunks = {}          # chunk idx -> staged aT tile
            msg_of = {}          # block -> msg pair tile [P, 2, H]
            Ss_of = {}           # block -> one-hot S pair tile [P, 2, P]

            def stage_chunk(ci):
                if ci >= n_chunks or ci in chunks:
                    return
                t_ = sa.tile([P, CH * 2 * P], bf16, tag="aTc")
                lo = ci * CH * 2 * P
                hi = min((ci + 1) * CH * 2 * P, NBLK * 2 * P)
                nc.sync.dma_start(t_[:, : hi - lo], aT_d[:, lo:hi])
                chunks[ci] = t_

            state = {"b": 0, "tail": 0}

            def emit_block():
                blk = state["b"]
                if blk >= NBLK:
                    return
                state["b"] = blk + 1
                ci, off = blk // CH, (blk % CH) * 2 * P
                if blk % CH == 0:
                    stage_chunk(ci + 1)
                aTc = chunks[ci]
                full = blk_kind[blk] == 1
                Ss = sS.tile([P, 2, P], bf16, tag="Ss")
                nc.sync.dma_start(Ss[:], S_d[blk])
                Ss_of[blk] = Ss
                mp = smsg.tile([P, 2, H], bf16, tag="msg")
                for t_in in range(2 if full else 1):
                    sl = slice(off + t_in * P, off + (t_in + 1) * P)
                    zA = zp.tile([P, 512], f32, tag="z")
                    zB = zp.tile([P, 512], f32, tag="z")
                    nc.tensor.matmul(zA[:], aTc[:, sl], W1bd[:, :512],
                                     start=True, stop=True)
                    nc.tensor.matmul(zB[:], aTc[:, sl], W1bd[:, 512:],
                                     start=True, stop=True)
                    nc.scalar.activation(mp[:, t_in, :512], zA[:], Relu)
                    nc.vector.tensor_scalar_max(mp[:, t_in, 512:], zB[:], 0.0)
                msg_of[blk] = mp

            def emit_block_if(target):
                if state["b"] < min(target, NBLK):
                    emit_block()

            # prologue: window 0's blocks first, then the bulk constants
            # (W2 isn't needed until the first h2, ~10us in)
            stage_chunk(0)
            while state["b"] < int(base_blk[1]):
                emit_block()
            nc.sync.dma_start(W2s[:], W2_d[:].rearrange("c p f -> p c f"))
            nc.sync.dma_start(b2s[:], b2_d[:])

            Tt = None
            h2b = None
            for w in range(nw):
                if w % 2 == 0:
                    Tt = sT.tile([P, 2, G], fp8, tag="Tt")
                    nc.sync.dma_start(Tt[:], T_d[w // 2])
                    h2b = sh2.tile([P, 2, H], fp8, tag="h2b")
                nb = int(nblk_w[w])
                b0 = int(base_blk[w])
                target = int(base_blk[min(w + 2, nw)])

                # dual-form bf16 aggregation: g2T[f,dst] += msg[e,f].T @ S[e,dst]
                # (per tile slot; a tail block's unused slot 1 is skipped)
                slots = []
                for b in range(nb):
                    ns = 2 if blk_kind[b0 + b] == 1 else 1
                    slots += [(b0 + b, s_) for s_ in range(ns)]
                g2T = sg2T.tile([P, 8, P], fp8, tag="g2T")
                for p4 in range(4):
                    gA = gp.tile([P, 512], f32, tag="g")
                    gB = gp.tile([P, 512], f32, tag="g")
                    jA, jB = 2 * p4, 2 * p4 + 1
                    for si, (blk, s_) in enumerate(slots):
                        st_, sp_ = si == 0, si == len(slots) - 1
                        nc.tensor.matmul(
                            gA[:, :P], msg_of[blk][:, s_, jA * P : (jA + 1) * P],
                            Ss_of[blk][:, s_, :], start=st_, stop=sp_,
                        )
                        nc.tensor.matmul(
                            gB[:, :P], msg_of[blk][:, s_, jB * P : (jB + 1) * P],
                            Ss_of[blk][:, s_, :], start=st_, stop=sp_,
                        )
                    nc.scalar.activation(g2T[:, jA], gA[:, :P], Copy, scale=GS)
                    nc.vector.tensor_scalar_mul(g2T[:, jB], gB[:, :P], GS)
                    if p4 < 3:
                        emit_block_if(target)   # cover gp drain w/ L1 stream

                # h2 = relu((g2*GS @ W2*16)/128 + b2); fp8 DoubleRow pairs
                hps = []
                for half in range(2):
                    lo = half * 512
                    h2p = hp.tile([P, 512], f32, tag="h")
                    for j2 in range(4):
                        nc.tensor.matmul(
                            h2p[:], g2T[:, 2 * j2 : 2 * j2 + 2, :],
                            W2s[:, 2 * j2 : 2 * j2 + 2, lo : lo + 512],
                            start=(j2 == 0), stop=False, perf_mode=DR,
                        )
                    nc.tensor.matmul(
                        h2p[:], ones1[:1, :], b2s[:1, lo : lo + 512],
                        start=False, stop=True,
                    )
                    hps.append(h2p)
                    if half == 0:
                        emit_block_if(target)
                # h2 stored fp8 x16 into this window's slot of the pair tile
                sl2 = w % 2
                nc.scalar.activation(h2b[:, sl2, :512], hps[0][:], Relu,
                                     scale=16.0 / 128)
                nc.scalar.activation(h2b[:, sl2, 512:], hps[1][:], Relu,
                                     scale=16.0 / 128)
                emit_block_if(target)
                if w % 2 == 1:
                    # pg3 += sum over the window pair: T.T @ h2  (fp8 DR)
                    for half in range(2):
                        lo = half * 512
                        cp = hp.tile([P, 512], f32, tag="h")
                        nc.tensor.matmul(
                            cp[:], Tt[:], h2b[:, :, lo : lo + 512],
                            start=True, stop=True, perf_mode=DR,
                        )
                        nc.vector.tensor_add(
                            pg3s[:, lo : lo + 512], pg3s[:, lo : lo + 512], cp[:]
                        )
                for b in range(nb):
                    msg_of.pop(b0 + b, None)
                    Ss_of.pop(b0 + b, None)

            nc.sync.dma_start(out_d[:], pg3s[:])

    nc.finalize()
    return nc


def kernel(x, W1, b1, W2, b2, W3, b3, Wlin, blin, edge_index, batch, num_graphs):
    import ml_dtypes
    from concourse.bass_utils import run_bass_kernel_spmd

    bf16 = ml_dtypes.bfloat16
    fp8 = ml_dtypes.float8_e4m3
    x = np.asarray(x, dtype=np.float32)
    W1 = np.asarray(W1, dtype=np.float32)
    b1 = np.asarray(b1, dtype=np.float32)
    W2 = np.asarray(W2, dtype=np.float32)
    b2 = np.asarray(b2, dtype=np.float32)
    W3 = np.asarray(W3, dtype=np.float32)
    b3 = np.asarray(b3, dtype=np.float32)
    Wlin = np.asarray(Wlin, dtype=np.float32)
    blin = np.asarray(blin, dtype=np.float32)

    (aT4, S2, Tpad, cnt, nblk_w, blk_kind, NBLK, base_blk) = _host_prep(
        x, edge_index, batch
    )

    nc = _build_device_program(NBLK, nblk_w, blk_kind, base_blk)

    W1b = np.zeros((4, H), dtype=np.float32)
    W1b[:2] = W1
    W1b[2] = b1
    W1b = np.tile(W1b / 32.0, (32, 1)).astype(bf16)   # [128, H]
    W2r = np.ascontiguousarray((W2 * 16.0).reshape(8, P, H)).astype(fp8)
    b2r = (b2 * 128.0).reshape(1, H).astype(bf16)

    in_maps = [
        {
            "aT": np.ascontiguousarray(aT4[c]),
            "S": np.ascontiguousarray(S2[c]),
            "T": np.ascontiguousarray(Tpad[c]),
            "W1b": W1b,
            "W2": W2r,
            "b2": b2r,
        }
        for c in range(N_CORES)
    ]
    res = run_bass_kernel_spmd(nc, in_maps, core_ids=list(range(N_CORES)))
    global LAST_RESULTS
    LAST_RESULTS = res
    pg3 = np.zeros((G, H), dtype=np.float64)
    for r in res.results:
        pg3 += r["pg3"].astype(np.float64)
    pg3 = (pg3 / 128.0).astype(np.float32)   # undo fp8 T(x8) / h2(x16) scales

    pooled = (pg3 @ W3 + cnt[:, None] * b3[None, :]) / np.maximum(cnt, 1.0)[:, None]
    out = pooled @ Wlin + blin[None, :]
    return out.astype(np.float32)
